# revision 1
# baseline (speedup 1.0000x reference)
"""Multi-head attention (B=4, T=2048, C=1024, H=16, D=64) on 8 TRN2 cores.

Sharding: core i handles batch b=i//2 and the 8 heads of half hh=i%2.
Each core computes its heads' contribution through the row-sharded output
projection -> partial y [T, C]; host sums the two partials per batch.

Per-core layouts (host pre-arranged):
  xT  [C, T]    = x[b].T
  wq/wk/wv [C, 512]  columns = (local head)*64 + d
  wpt [512, C]  rows  = (local head)*64 + d   (= Wp.T row-slice)
  bp  [C]       bias on even cores, zeros on odd (summed partials)

On-chip dataflow per core:
  qT/kT [2h*64=128, T] via lhsT=w-chunk, rhs=xT-chunk (f32r, N=512)
  v     [s,d] natural via lhsT=xT s-slice, rhs=wv-chunk (N=256)
  scoresT[s,t]: lhsT=kT s-block [64,128], rhs=qT t-tile [64,512],
                2 heads row-tiled (K=64 each, partitions 0-63 / 64-127)
  exp on ScalarE PSUM->SBUF with scale=1/sqrt(C); causal: restrict to the
  valid t-range, one constant [128,128] mask multiply on straddling blocks
  PV: lhsT=[v ; ones] [128,65], rhs=pT -> outT [65,512] PSUM accumulated
  over s-blocks; row 64 = softmax normalizer Z
  normalize: DVE reciprocal(Z) -> gpsimd partition_broadcast -> DVE mult
  y: lhsT=outcatT [c,t-block], rhs=wpt [c, c'] + bias, DMA out
"""

import os
import sys

import numpy as np

for _p in ("/opt/trn_rl_repo", "/root/.axon_site/_ro/trn_rl_repo"):
    if os.path.isdir(_p) and _p not in sys.path:
        sys.path.append(_p)

import concourse.bass as bass
import concourse.bacc as bacc
import concourse.mybir as mybir
import concourse.tile as tile
from concourse.bass_utils import run_bass_kernel_spmd

B, T, C, H, D = 4, 2048, 1024, 16, 64
HL = H // 2          # heads per core
P = 128
NCH = C // P         # 8 c-chunks
NTT = T // 512       # 4 t-tiles of 512
NSB = T // P         # 16 s-blocks of 128
SCALE = 1.0 / 32.0   # 1/sqrt(C)

F32 = mybir.dt.float32
F32R = mybir.dt.float32r


def _build(causal: bool, debug: bool = False) -> bass.Bass:
    nc = bacc.Bacc("TRN2", target_bir_lowering=False, debug=False, num_devices=8)

    xT = nc.dram_tensor("xT", [C, T], F32R, kind="ExternalInput").ap()
    wq_d = nc.dram_tensor("wq", [C, HL * D], F32R, kind="ExternalInput").ap()
    wk_d = nc.dram_tensor("wk", [C, HL * D], F32R, kind="ExternalInput").ap()
    wv_d = nc.dram_tensor("wv", [C, HL * D], F32R, kind="ExternalInput").ap()
    wpt_d = nc.dram_tensor("wpt", [HL * D, C], F32R, kind="ExternalInput").ap()
    bp_d = nc.dram_tensor("bp", [C], F32, kind="ExternalInput").ap()
    y_d = nc.dram_tensor("y", [T, C], F32, kind="ExternalOutput").ap()
    dbg = {}
    if debug:
        dbg["q"] = nc.dram_tensor("dbg_q", [2, P, T], F32, kind="ExternalOutput").ap()
        dbg["k"] = nc.dram_tensor("dbg_k", [2, P, T], F32, kind="ExternalOutput").ap()
        dbg["v"] = nc.dram_tensor("dbg_v", [P, NSB * 4 * (D + 1)], F32, kind="ExternalOutput").ap()
        dbg["oc"] = nc.dram_tensor("dbg_oc", [4, P, T], F32, kind="ExternalOutput").ap()

    with tile.TileContext(nc) as tc:
        _emit(nc, tc, causal, xT, wq_d, wk_d, wv_d, wpt_d, bp_d, y_d, dbg)
    nc.compile()
    return nc


def _emit(nc, tc, causal, xT, wq_d, wk_d, wv_d, wpt_d, bp_d, y_d, dbg={}):
    from contextlib import ExitStack

    ctx = ExitStack()
    with ctx:
        consts = ctx.enter_context(tc.tile_pool(name="consts", bufs=1))
        q_pool = ctx.enter_context(tc.tile_pool(name="qT", bufs=3))
        k_pool = ctx.enter_context(tc.tile_pool(name="kT", bufs=3))
        v_pool = ctx.enter_context(tc.tile_pool(name="v", bufs=2))
        oc_pool = ctx.enter_context(tc.tile_pool(name="outcat", bufs=4))
        p_pool = ctx.enter_context(tc.tile_pool(name="pT", bufs=3))
        z_pool = ctx.enter_context(tc.tile_pool(name="zb", bufs=2))
        rzb_pool = ctx.enter_context(tc.tile_pool(name="rzb", bufs=2))
        psA = ctx.enter_context(tc.tile_pool(name="psA", bufs=2, space="PSUM"))
        psB = ctx.enter_context(tc.tile_pool(name="psB", bufs=2, space="PSUM"))
        pso = ctx.enter_context(tc.tile_pool(name="pso", bufs=2, space="PSUM"))

        # constant [128, 2, 128] additive causal mask: 0 where free>=partition
        # else -1e9 (two copies along the middle dim, one per row-tiled head)
        mask = None
        if causal:
            mask = consts.tile([P, 2, P], F32)
            nc.vector.memset(mask, 0.0)
            for _u in range(2):
                nc.gpsimd.affine_select(
                    out=mask[:, _u, :], in_=mask[:, _u, :],
                    compare_op=mybir.AluOpType.is_ge,
                    fill=-1e9, base=0,
                    pattern=[[1, P]], channel_multiplier=-1,
                )

        ones_bc = consts.tile([P, P], F32R)
        nc.vector.memset(ones_bc.bitcast(F32), 1.0)

        outcat = [oc_pool.tile([P, T], F32R, tag="outcat", name=f"outcat{i}")
                  for i in range(4)]

        inner = ExitStack()
        with inner:
            wq_pool = inner.enter_context(tc.tile_pool(name="wq", bufs=1))
            wk_pool = inner.enter_context(tc.tile_pool(name="wk", bufs=1))
            wv_pool = inner.enter_context(tc.tile_pool(name="wv", bufs=1))
            x_pool = inner.enter_context(tc.tile_pool(name="xh", bufs=1))

            for hg in range(2):  # head-group of 4 heads (= 2 pairs)
                hsl = slice(hg * 4 * D, (hg + 1) * 4 * D)
                wq_t = wq_pool.tile([P, NCH, 4 * D], F32R, tag="wq")
                wk_t = wk_pool.tile([P, NCH, 4 * D], F32R, tag="wk")
                wv_t = wv_pool.tile([P, NCH, 4 * D], F32R, tag="wv")
                for w_t, w_d in ((wq_t, wq_d), (wk_t, wk_d), (wv_t, wv_d)):
                    nc.sync.dma_start(
                        out=w_t,
                        in_=w_d[:, hsl].rearrange("(n p) d -> p n d", p=P),
                    )

                qT2 = [q_pool.tile([P, T], F32R, tag="qT", name=f"qT{i}")
                       for i in range(2)]
                kT2 = [k_pool.tile([P, T], F32R, tag="kT", name=f"kT{i}")
                       for i in range(2)]
                # v: [s-part, s-block, head-in-group, d + ones]
                v_t = v_pool.tile([P, NSB, 4, D + 1], F32R, tag="v")
                nc.vector.memset(v_t[:, :, :, D:].bitcast(F32), 1.0)

                for th in range(2):  # t/s halves of 1024
                    xh = x_pool.tile([P, NCH, 1024], F32R, tag="xh")
                    for c in range(NCH):
                        nc.sync.dma_start(
                            out=xh[:, c, :],
                            in_=xT[c * P:(c + 1) * P, th * 1024:(th + 1) * 1024],
                        )
                    tg = slice(th * 1024, (th + 1) * 1024)
                    # ---- q/k projections ----
                    for pr in range(2):
                        wsl = slice(pr * P, (pr + 1) * P)
                        qps = psA.tile([P, 2, 512], F32, tag="psA", name="qps")
                        kps = psA.tile([P, 2, 512], F32, tag="psA", name="kps")
                        for c in range(NCH):
                            for tt in range(2):
                                nc.tensor.matmul(
                                    qps[:, tt, :], wq_t[:, c, wsl],
                                    xh[:, c, tt * 512:(tt + 1) * 512],
                                    start=c == 0, stop=c == NCH - 1)
                                nc.tensor.matmul(
                                    kps[:, tt, :], wk_t[:, c, wsl],
                                    xh[:, c, tt * 512:(tt + 1) * 512],
                                    start=c == 0, stop=c == NCH - 1)
                        nc.vector.tensor_copy(
                            out=qT2[pr][:, tg],
                            in_=qps.rearrange("p u t -> p (u t)"))
                        nc.vector.tensor_copy(
                            out=kT2[pr][:, tg],
                            in_=kps.rearrange("p u t -> p (u t)"))
                    # ---- v projection (natural [s, d]) ----
                    for sbp in range(4):
                        vps = psB.tile([P, 2, 256], F32, tag="psB", name="vps")
                        for c in range(NCH):
                            for u in range(2):
                                nc.tensor.matmul(
                                    vps[:, u, :],
                                    xh[:, c, (sbp * 2 + u) * P:(sbp * 2 + u + 1) * P],
                                    wv_t[:, c, :],
                                    start=(c == 0 and u == 0), stop=c == NCH - 1)
                        sb0 = th * 8 + sbp * 2
                        nc.vector.tensor_copy(
                            out=v_t[:, sb0:sb0 + 2, :, 0:D],
                            in_=vps.rearrange("p u (h d) -> p u h d", h=4),
                        )

                if dbg and hg == 0:
                    for pr2 in range(2):
                        nc.sync.dma_start(out=dbg["q"][pr2], in_=qT2[pr2].bitcast(F32))
                        nc.sync.dma_start(out=dbg["k"][pr2], in_=kT2[pr2].bitcast(F32))
                    nc.sync.dma_start(
                        out=dbg["v"],
                        in_=v_t.rearrange("p a b c -> p (a b c)").bitcast(F32))

                # ---- attention for this head-group ----
                for pr in range(2):
                    pair = hg * 2 + pr
                    zb = z_pool.tile([P, 3, 512], F32, tag="zb", name="zb")
                    nc.vector.memset(zb, 1.0)
                    for j in range(NTT):
                        nsb_j = 4 * (j + 1) if causal else NSB
                        outp = [pso.tile([D + 1, 512], F32, tag="pso",
                                         name=f"outp{i}") for i in range(2)]
                        def emit_pv(i, lo, last):
                            for u in range(2):
                                nc.tensor.matmul(
                                    outp[u][:, lo:512],
                                    v_t[:, i, pr * 2 + u, :],
                                    pend[i][:, u, lo:512],
                                    start=(i == 0), stop=last,
                                    skip_group_check=True)
                            del pend[i]

                        pend = {}
                        prev = None
                        for i in range(nsb_j):
                            r = i - 4 * j if causal else -1
                            lo = max(r, 0) * P
                            last = i == nsb_j - 1
                            scs = psA.tile([P, 2, 512], F32, tag="psA", name="scs")
                            pts = p_pool.tile([P, 2, 512], F32R, tag="pT", name="pts")
                            pend[i] = pts
                            for u in range(2):
                                dsl = slice(u * D, (u + 1) * D)
                                nc.tensor.matmul(
                                    scs[:, u, :],
                                    kT2[pr][dsl, i * P:(i + 1) * P],
                                    qT2[pr][dsl, j * 512:(j + 1) * 512],
                                    start=True, stop=True)
                            if causal and r >= 0:
                                nc.vector.tensor_add(
                                    scs[:, :, lo:lo + P],
                                    scs[:, :, lo:lo + P],
                                    mask)
                            nc.scalar.activation(
                                out=pts[:, :, lo:512],
                                in_=scs[:, :, lo:512],
                                func=mybir.ActivationFunctionType.Exp,
                                scale=SCALE)
                            if prev is not None:
                                emit_pv(*prev)
                            prev = (i, lo, last)
                        if prev is not None:
                            emit_pv(*prev)
                        for u in range(2):
                            # raw (unnormalized) head output + Z row gather
                            nc.vector.tensor_copy(
                                out=outcat[pair][u * D:(u + 1) * D,
                                                 j * 512:(j + 1) * 512],
                                in_=outp[u][0:D, :])
                            idx = j * 2 + u
                            nc.vector.tensor_copy(
                                out=zb[32 * (idx // 3):32 * (idx // 3) + 1,
                                       idx % 3, :],
                                in_=outp[u][D:D + 1, :])
                    # batched normalizer: one reciprocal for all 8 (j, u)
                    # rows, then per-row broadcast via K=1 matmul into PSUM
                    rzb_all = z_pool.tile([P, 3, 512], F32R, tag="zb", name="rz_all")
                    with nc.allow_low_precision(reason="softmax normalizer"):
                        nc.vector.reciprocal(out=rzb_all, in_=zb)
                    for j in range(NTT):
                        for u in range(2):
                            idx = j * 2 + u
                            k0 = 32 * (idx // 3)
                            bps = pso.tile([P, 512], F32, tag="pso", name="bps")
                            nc.tensor.matmul(
                                bps,
                                ones_bc[k0:k0 + 1, :],
                                rzb_all[k0:k0 + 1, idx % 3, :],
                                start=True, stop=True)
                            osl = outcat[pair][u * D:(u + 1) * D,
                                               j * 512:(j + 1) * 512]
                            nc.vector.tensor_mul(
                                osl, osl.bitcast(F32),
                                bps[u * D:(u + 1) * D, :])

        if dbg:
            for q2 in range(4):
                nc.sync.dma_start(out=dbg["oc"][q2], in_=outcat[q2].bitcast(F32))

        # ---- output projection ----
        wpt_pool = ctx.enter_context(tc.tile_pool(name="wpt", bufs=4))
        bpb_pool = ctx.enter_context(tc.tile_pool(name="bpb", bufs=1))
        yst_pool = ctx.enter_context(tc.tile_pool(name="yst", bufs=3))
        wpt_t = [wpt_pool.tile([P, C], F32R, tag="wpt", name=f"wpt{i}") for i in range(4)]
        for q in range(4):
            nc.sync.dma_start(out=wpt_t[q], in_=wpt_d[q * P:(q + 1) * P, :])
        bpb = bpb_pool.tile([P, C], F32)
        nc.sync.dma_start(
            out=bpb,
            in_=bass.AP(tensor=bp_d.tensor, offset=0, ap=[[0, P], [1, C]]),
        )
        for m in range(T // P):
            for n in range(2):
                yps = psB.tile([P, 512], F32, tag="psB", name="yps")
                for q in range(4):
                    nc.tensor.matmul(
                        yps,
                        outcat[q][:, m * P:(m + 1) * P],
                        wpt_t[q][:, n * 512:(n + 1) * 512],
                        start=(q == 0), stop=(q == 3))
                yt = yst_pool.tile([P, 512], F32, tag="yst", name="yt")
                nc.vector.tensor_add(yt, yps, bpb[:, n * 512:(n + 1) * 512])
                nc.sync.dma_start(
                    out=y_d[m * P:(m + 1) * P, n * 512:(n + 1) * 512],
                    in_=yt)


_NC_CACHE = {}
LAST_RESULTS = None


def kernel(x, Wq, Wk, Wv, Wp, bp, is_masked, **_unused):
    global LAST_RESULTS
    x = np.asarray(x, np.float32)
    Wq = np.asarray(Wq, np.float32)
    Wk = np.asarray(Wk, np.float32)
    Wv = np.asarray(Wv, np.float32)
    Wp = np.asarray(Wp, np.float32)
    bp = np.asarray(bp, np.float32)
    causal = bool(np.asarray(is_masked).item())

    if causal not in _NC_CACHE:
        _NC_CACHE[causal] = _build(causal)
    nc = _NC_CACHE[causal]

    # host-side layout prep
    wq_r = np.ascontiguousarray(Wq.transpose(1, 0, 2).reshape(C, H * D))
    wk_r = np.ascontiguousarray(Wk.transpose(1, 0, 2).reshape(C, H * D))
    wv_r = np.ascontiguousarray(Wv.transpose(1, 0, 2).reshape(C, H * D))
    wpt = np.ascontiguousarray(Wp.T)
    zeros = np.zeros_like(bp)

    xTs = [np.ascontiguousarray(x[b].T) for b in range(B)]
    in_maps = []
    for core in range(8):
        b, hh = core // 2, core % 2
        csl = slice(hh * HL * D, (hh + 1) * HL * D)
        in_maps.append({
            "xT": xTs[b],
            "wq": np.ascontiguousarray(wq_r[:, csl]),
            "wk": np.ascontiguousarray(wk_r[:, csl]),
            "wv": np.ascontiguousarray(wv_r[:, csl]),
            "wpt": np.ascontiguousarray(wpt[csl, :]),
            "bp": bp if hh == 0 else zeros,
        })

    trace = bool(int(os.environ.get("KERNEL_TRACE", "0")))
    res = run_bass_kernel_spmd(
        nc, in_maps, core_ids=list(range(8)), trace=trace)
    LAST_RESULTS = res

    y = np.empty((B, T, C), np.float32)
    for b in range(B):
        y[b] = res.results[2 * b]["y"] + res.results[2 * b + 1]["y"]
    return y



# revision 8
# speedup vs baseline: 1.4845x; 1.4845x over previous
"""Multi-head attention (B=4, T=2048, C=1024, H=16, D=64) on 8 TRN2 cores.

Sharding: core i handles batch b=i//2 and the 8 heads of half hh=i%2.
Each core computes its heads' contribution through the row-sharded output
projection -> partial y [T, C]; host sums the two partials per batch.

v2: all matmul operands in bf16 (fp32 "HIGH-mode" matmuls run at ~half PE
rate and block FastWeightLoad), x resident in SBUF once, projections for
all 8 local heads up front (pair-major q/k, s-major v), per-pair
normalization deferred one pair so the reciprocal never stalls the PE,
reciprocal_approx_fast instead of the slow exact reciprocal.

Per-core layouts (host pre-arranged, bf16):
  xT  [C, T]    = x[b].T
  wq/wk/wv [C, 512]  columns = (local head)*64 + d
  wpt [512, C]  rows  = (local head)*64 + d   (= Wp.T row-slice)
  bp  [C] f32   bias on even cores, zeros on odd (summed partials)

On-chip dataflow per core:
  qT/kT [128, T] per head-pair via lhsT=w-chunk, rhs=xT-chunk (N=512)
  v     [s, h, d] natural via lhsT=xT s-slice, rhs=wv (N=512, all 8 heads)
  scoresT[s,t]: lhsT=kT s-block [64,128], rhs=qT t-tile [64,<=512],
                2 heads row-tiled (K=64 each, partitions 0-63 / 64-127,
                concurrent on the PE array)
  exp on ScalarE PSUM->SBUF bf16 with scale=1/sqrt(C); causal via additive
  -1e9 mask on the straddling 128-blocks
  PV: lhsT=[v ; ones] [128,65] bf16, rhs=pT -> outT [65,2,512] PSUM
  accumulated over s-blocks; row 64 = softmax normalizer Z
  normalize: reciprocal_approx_fast(Z) -> bf16 -> per-row K=1 matmul
  broadcast -> DVE mult into outcat (deferred one pair)
  y: lhsT=outcat [c,t-block], rhs=wpt [c, c'] + bias, DMA out
"""

import os
import sys

import numpy as np

for _p in ("/opt/trn_rl_repo", "/root/.axon_site/_ro/trn_rl_repo"):
    if os.path.isdir(_p) and _p not in sys.path:
        sys.path.append(_p)

import concourse.bass as bass
import concourse.bacc as bacc
import concourse.mybir as mybir
import concourse.tile as tile
from concourse.bass_utils import run_bass_kernel_spmd

B, T, C, H, D = 4, 2048, 1024, 16, 64
HL = H // 2          # heads per core
P = 128
NCH = C // P         # 8 c-chunks
NTT = T // 512       # 4 t-tiles of 512
NSB = T // P         # 16 s-blocks of 128
SCALE = 1.0 / 32.0   # 1/sqrt(C)

F32 = mybir.dt.float32
BF16 = mybir.dt.bfloat16


def _build(causal: bool) -> bass.Bass:
    nc = bacc.Bacc("TRN2", target_bir_lowering=False, debug=False, num_devices=8)

    xT = nc.dram_tensor("xT", [C, T], BF16, kind="ExternalInput").ap()
    wq_d = nc.dram_tensor("wq", [C, HL * D], BF16, kind="ExternalInput").ap()
    wk_d = nc.dram_tensor("wk", [C, HL * D], BF16, kind="ExternalInput").ap()
    wv_d = nc.dram_tensor("wv", [C, HL * D], BF16, kind="ExternalInput").ap()
    wpt_d = nc.dram_tensor("wpt", [HL * D, C], BF16, kind="ExternalInput").ap()
    bp_d = nc.dram_tensor("bp", [C], F32, kind="ExternalInput").ap()
    y_d = nc.dram_tensor("y", [T, C], F32, kind="ExternalOutput").ap()

    with tile.TileContext(nc) as tc:
        _emit(nc, tc, causal, xT, wq_d, wk_d, wv_d, wpt_d, bp_d, y_d)
    nc.compile()
    return nc


def _emit(nc, tc, causal, xT, wq_d, wk_d, wv_d, wpt_d, bp_d, y_d):
    from contextlib import ExitStack

    ctx = ExitStack()
    with ctx:
        consts = ctx.enter_context(tc.tile_pool(name="consts", bufs=1))
        x_pool = ctx.enter_context(tc.tile_pool(name="xh", bufs=1))
        w_pool = ctx.enter_context(tc.tile_pool(name="w", bufs=1))
        q_pool = ctx.enter_context(tc.tile_pool(name="qT", bufs=3))
        k_pool = ctx.enter_context(tc.tile_pool(name="kT", bufs=3))
        v_pool = ctx.enter_context(tc.tile_pool(name="v", bufs=1))
        oc_pool = ctx.enter_context(tc.tile_pool(name="outcat", bufs=4))
        p_pool = ctx.enter_context(tc.tile_pool(name="pT", bufs=3))
        z_pool = ctx.enter_context(tc.tile_pool(name="zb", bufs=2))
        wpt_pool = ctx.enter_context(tc.tile_pool(name="wpt", bufs=4))
        bpb_pool = ctx.enter_context(tc.tile_pool(name="bpb", bufs=1))
        yst_pool = ctx.enter_context(tc.tile_pool(name="yst", bufs=3))
        psA = ctx.enter_context(tc.tile_pool(name="psA", bufs=2, space="PSUM"))
        psO = ctx.enter_context(tc.tile_pool(name="psO", bufs=2, space="PSUM"))

        # ---- constants ----
        # additive causal mask: 0 where free>=partition else -1e9
        mask = None
        if causal:
            mask = consts.tile([P, 2, P], F32)
            nc.vector.memset(mask, 0.0)
            for _u in range(2):
                nc.gpsimd.affine_select(
                    out=mask[:, _u, :], in_=mask[:, _u, :],
                    compare_op=mybir.AluOpType.is_ge,
                    fill=-1e9, base=0,
                    pattern=[[1, P]], channel_multiplier=-1,
                )
        ones_bc = consts.tile([P, P], BF16)
        nc.vector.memset(ones_bc, 1.0)

        # ---- weights + x DMA (weights first: first matmuls need them) ----
        wq_t = w_pool.tile([P, NCH, HL * D], BF16, tag="wq", name="wq")
        wk_t = w_pool.tile([P, NCH, HL * D], BF16, tag="wk", name="wk")
        wv_t = w_pool.tile([P, NCH, HL * D], BF16, tag="wv", name="wv")
        for w_t, w_d in ((wq_t, wq_d), (wk_t, wk_d), (wv_t, wv_d)):
            nc.sync.dma_start(
                out=w_t, in_=w_d.rearrange("(n p) d -> p n d", p=P))

        xh = x_pool.tile([P, NCH, T], BF16, tag="xh")
        for th in range(2):
            tg = slice(th * 1024, (th + 1) * 1024)
            nc.sync.dma_start(
                out=xh[:, :, tg],
                in_=xT[:, tg].rearrange("(n p) t -> p n t", p=P))

        wpt_t = [wpt_pool.tile([P, C], BF16, tag="wpt", name=f"wpt{i}")
                 for i in range(4)]
        for q in range(4):
            nc.sync.dma_start(out=wpt_t[q], in_=wpt_d[q * P:(q + 1) * P, :])
        bpb = bpb_pool.tile([P, C], F32)
        nc.sync.dma_start(
            out=bpb,
            in_=bass.AP(tensor=bp_d.tensor, offset=0, ap=[[0, P], [1, C]]))

        # ---- persistent activation tiles ----
        qT = [q_pool.tile([P, T], BF16, tag="qT", name=f"qT{i}")
              for i in range(4)]
        kT = [k_pool.tile([P, T], BF16, tag="kT", name=f"kT{i}")
              for i in range(4)]
        # v: [s-part, s-block, head, d + ones]
        v_t = v_pool.tile([P, NSB, HL, D + 1], BF16, tag="v")
        nc.vector.memset(v_t[:, :, :, D:], 1.0)
        outcat = [oc_pool.tile([P, T], BF16, tag="outcat", name=f"outcat{i}")
                  for i in range(4)]

        def qk_proj(pr):
            wsl = slice(pr * P, (pr + 1) * P)
            for th in range(2):
                qps = psA.tile([P, 2, 512], F32, tag="psA", name="qps")
                kps = psA.tile([P, 2, 512], F32, tag="psA", name="kps")
                for c in range(NCH):
                    for w_t, ps in ((wq_t, qps), (wk_t, kps)):
                        for tt in range(2):
                            t0 = th * 1024 + tt * 512
                            nc.tensor.matmul(
                                ps[:, tt, :], w_t[:, c, wsl],
                                xh[:, c, t0:t0 + 512],
                                start=c == 0, stop=c == NCH - 1)
                tg = slice(th * 1024, (th + 1) * 1024)
                nc.vector.tensor_copy(
                    out=qT[pr][:, tg], in_=qps.rearrange("p u t -> p (u t)"))
                nc.vector.tensor_copy(
                    out=kT[pr][:, tg], in_=kps.rearrange("p u t -> p (u t)"))

        def v_proj(s0, s1):
            for s in range(s0, s1):
                vps = psO.tile([P, 512], F32, tag="psO", name="vps")
                for c in range(NCH):
                    nc.tensor.matmul(
                        vps, xh[:, c, s * P:(s + 1) * P], wv_t[:, c, :],
                        start=c == 0, stop=c == NCH - 1)
                nc.vector.tensor_copy(
                    out=v_t[:, s:s + 1, :, 0:D],
                    in_=vps.rearrange("p (o h d) -> p o h d", o=1, h=HL))

        def attention(pair, pre_j=None):
            # zb row (j,u) -> idx=2j+u at partition 32*(idx//3), slot idx%3
            # (base partitions are restricted to {0,32,64})
            zb = z_pool.tile([P, 3, 512], F32, tag="zb", name=f"zb{pair}")
            for j in range(NTT):
                if pre_j is not None:
                    pre_j(j)
                nsb_j = 4 * (j + 1) if causal else NSB
                outp = psO.tile([D + 1, 2, 512], F32, tag="psO", name="outp")

                def emit_pv(i, lo, last):
                    for u in range(2):
                        nc.tensor.matmul(
                            outp[:, u, lo:512],
                            v_t[:, i, pair * 2 + u, :],
                            pend[i][:, u, lo:512],
                            start=(i == 0), stop=last,
                            skip_group_check=True)
                    del pend[i]

                pend = {}
                prev = None
                for i in range(nsb_j):
                    r = i - 4 * j if causal else -1
                    lo = max(r, 0) * P
                    last = i == nsb_j - 1
                    scs = psA.tile([P, 2, 512], F32, tag="psA", name="scs")
                    pts = p_pool.tile([P, 2, 512], BF16, tag="pT", name="pts")
                    pend[i] = pts
                    for u in range(2):
                        dsl = slice(u * D, (u + 1) * D)
                        nc.tensor.matmul(
                            scs[:, u, lo:512],
                            kT[pair][dsl, i * P:(i + 1) * P],
                            qT[pair][dsl, j * 512 + lo:(j + 1) * 512],
                            start=True, stop=True)
                    if causal and r >= 0:
                        nc.vector.tensor_add(
                            scs[:, :, lo:lo + P], scs[:, :, lo:lo + P], mask)
                    nc.scalar.activation(
                        out=pts[:, :, lo:512], in_=scs[:, :, lo:512],
                        func=mybir.ActivationFunctionType.Exp, scale=SCALE)
                    if prev is not None:
                        emit_pv(*prev)
                    prev = (i, lo, last)
                if prev is not None:
                    emit_pv(*prev)
                for u in range(2):
                    # raw (unnormalized) head output
                    nc.vector.tensor_copy(
                        out=outcat[pair][u * D:(u + 1) * D,
                                         j * 512:(j + 1) * 512],
                        in_=outp[0:D, u, :])
                # softmax normalizer rows -> zb
                for u in range(2):
                    idx = 2 * j + u
                    k0 = 32 * (idx // 3)
                    nc.vector.tensor_copy(
                        out=zb[k0:k0 + 1, idx % 3, :],
                        in_=outp[D:D + 1, u, :])
            return zb

        def normalize(pair, zb):
            rz = z_pool.tile([P, 3, 512], F32, tag="rz", name="rz")
            nc.vector.reciprocal_approx_fast(out=rz, in_=zb)
            rzb = z_pool.tile([P, 3, 512], BF16, tag="rzb", name="rzb")
            nc.vector.tensor_copy(out=rzb, in_=rz)
            for j in range(NTT):
                for u in range(2):
                    idx = 2 * j + u
                    k0 = 32 * (idx // 3)
                    bps = psO.tile([P, 512], F32, tag="psO", name="bps")
                    nc.tensor.matmul(
                        bps, ones_bc[k0:k0 + 1, :],
                        rzb[k0:k0 + 1, idx % 3, :],
                        start=True, stop=True)
                    osl = outcat[pair][u * D:(u + 1) * D,
                                       j * 512:(j + 1) * 512]
                    nc.vector.tensor_mul(osl, osl, bps[u * D:(u + 1) * D, :])

        # ---- schedule: proj(0) -> attn(0) | proj(p+1) -> attn(p+1),
        #      norm(p) deferred behind attn(p+1) so recip never stalls PE ----
        qk_proj(0)
        v_proj(0, 4)

        def pre_j0(j):
            if j > 0:
                v_proj(4 * j, 4 * (j + 1))

        zbs = [None] * 4
        zbs[0] = attention(0, pre_j=pre_j0)
        for pr in range(1, 4):
            qk_proj(pr)
            zbs[pr] = attention(pr)
            normalize(pr - 1, zbs[pr - 1])
        normalize(3, zbs[3])

        # ---- output projection ----
        for m in range(T // P):
            for n in range(2):
                yps = psO.tile([P, 512], F32, tag="psO", name="yps")
                for q in range(4):
                    nc.tensor.matmul(
                        yps,
                        outcat[q][:, m * P:(m + 1) * P],
                        wpt_t[q][:, n * 512:(n + 1) * 512],
                        start=(q == 0), stop=(q == 3))
                yt = yst_pool.tile([P, 512], F32, tag="yst", name="yt")
                nc.vector.tensor_add(yt, yps, bpb[:, n * 512:(n + 1) * 512])
                nc.sync.dma_start(
                    out=y_d[m * P:(m + 1) * P, n * 512:(n + 1) * 512],
                    in_=yt)


_NC_CACHE = {}
LAST_RESULTS = None


def kernel(x, Wq, Wk, Wv, Wp, bp, is_masked, **_unused):
    global LAST_RESULTS
    from ml_dtypes import bfloat16

    x = np.asarray(x, np.float32)
    Wq = np.asarray(Wq, np.float32)
    Wk = np.asarray(Wk, np.float32)
    Wv = np.asarray(Wv, np.float32)
    Wp = np.asarray(Wp, np.float32)
    bp = np.asarray(bp, np.float32)
    causal = bool(np.asarray(is_masked).item())

    if causal not in _NC_CACHE:
        _NC_CACHE[causal] = _build(causal)
    nc = _NC_CACHE[causal]

    # host-side layout prep
    wq_r = np.ascontiguousarray(Wq.transpose(1, 0, 2).reshape(C, H * D))
    wk_r = np.ascontiguousarray(Wk.transpose(1, 0, 2).reshape(C, H * D))
    wv_r = np.ascontiguousarray(Wv.transpose(1, 0, 2).reshape(C, H * D))
    wpt = np.ascontiguousarray(Wp.T)
    zeros = np.zeros_like(bp)

    xTs = [np.ascontiguousarray(x[b].T).astype(bfloat16) for b in range(B)]
    in_maps = []
    for core in range(8):
        b, hh = core // 2, core % 2
        csl = slice(hh * HL * D, (hh + 1) * HL * D)
        in_maps.append({
            "xT": xTs[b],
            "wq": np.ascontiguousarray(wq_r[:, csl]).astype(bfloat16),
            "wk": np.ascontiguousarray(wk_r[:, csl]).astype(bfloat16),
            "wv": np.ascontiguousarray(wv_r[:, csl]).astype(bfloat16),
            "wpt": np.ascontiguousarray(wpt[csl, :]).astype(bfloat16),
            "bp": bp if hh == 0 else zeros,
        })

    trace = bool(int(os.environ.get("KERNEL_TRACE", "0")))
    res = run_bass_kernel_spmd(
        nc, in_maps, core_ids=list(range(8)), trace=trace)
    LAST_RESULTS = res

    y = np.empty((B, T, C), np.float32)
    for b in range(B):
        y[b] = res.results[2 * b]["y"] + res.results[2 * b + 1]["y"]
    return y


# revision 9
# speedup vs baseline: 1.6149x; 1.0878x over previous
"""Multi-head attention (B=4, T=2048, C=1024, H=16, D=64) on 8 TRN2 cores.

Sharding: core i handles batch b=i//2 and the 8 heads of half hh=i%2.
Each core computes its heads' contribution through the row-sharded output
projection -> partial yT [C, T]; host transposes and sums the two partials
per batch.

v3: all matmul operands bf16 (fp32 "HIGH-mode" matmuls run at ~half PE rate
and block FastWeightLoad). The softmax exp on ScalarE (~157us) is the
per-pair bottleneck, so all PE work that is not on the exp critical path
(q/k projection quarters of the SAME pair one t-tile ahead, v projection
s-quarters, the deferred normalization of the PREVIOUS pair) is interleaved
into the attention j-loop where the PE otherwise idles waiting for exp.
Output projection emits yT = wpt_chunk.T @ outcat so the bias add becomes a
per-partition scalar on the (idle by then) ScalarE.

Per-core layouts (host pre-arranged, bf16):
  xT  [C, T]    = x[b].T
  wq/wk/wv [C, 512]  columns = (local head)*64 + d
  wpt [512, C]  rows  = (local head)*64 + d   (= Wp.T row-slice)
  bp  [C] f32   bias on even cores, zeros on odd (summed partials)
Output: yT [C, T] f32 (host transposes).

On-chip dataflow per core:
  qT/kT [128, T] per head-pair via lhsT=w-chunk, rhs=xT-chunk (N=512)
  v     [s, h, d] natural via lhsT=xT s-slice, rhs=wv (N=512, all 8 heads)
  scoresT[s,t]: lhsT=kT s-block [64,128], rhs=qT t-tile [64,<=512],
                2 heads row-tiled (K=64 each, concurrent on the PE array)
  exp on ScalarE PSUM->SBUF bf16 with scale=1/sqrt(C); causal via additive
  -1e9 mask on the straddling 128-blocks
  PV: lhsT=[v ; ones] [128,65] bf16, rhs=pT -> outT [65,512] PSUM per head,
  accumulated over s-blocks; row 64 = softmax normalizer Z
  normalize: reciprocal_approx_fast(Z) -> bf16 -> per-row K=1 matmul
  broadcast -> DVE mult into outcat (deferred one pair)
  yT: lhsT=wpt c'-chunk, rhs=outcat t-chunk; bias via ScalarE Identity
"""

import os
import sys

import numpy as np

for _p in ("/opt/trn_rl_repo", "/root/.axon_site/_ro/trn_rl_repo"):
    if os.path.isdir(_p) and _p not in sys.path:
        sys.path.append(_p)

import concourse.bass as bass
import concourse.bacc as bacc
import concourse.mybir as mybir
import concourse.tile as tile
from concourse.bass_utils import run_bass_kernel_spmd

B, T, C, H, D = 4, 2048, 1024, 16, 64
HL = H // 2          # heads per core
P = 128
NCH = C // P         # 8 c-chunks
NTT = T // 512       # 4 t-tiles of 512
NSB = T // P         # 16 s-blocks of 128
SCALE = 1.0 / 32.0   # 1/sqrt(C)

F32 = mybir.dt.float32
BF16 = mybir.dt.bfloat16

# zb row (j,u) -> idx=2j+u at partition 32*(idx//3), slot idx%3
# (AP base partitions are restricted to {0,32,64})
def _zslot(j, u):
    idx = 2 * j + u
    return 32 * (idx // 3), idx % 3


def _build(causal: bool) -> bass.Bass:
    nc = bacc.Bacc("TRN2", target_bir_lowering=False, debug=False, num_devices=8)

    xT = nc.dram_tensor("xT", [C, T], BF16, kind="ExternalInput").ap()
    wq_d = nc.dram_tensor("wq", [C, HL * D], BF16, kind="ExternalInput").ap()
    wk_d = nc.dram_tensor("wk", [C, HL * D], BF16, kind="ExternalInput").ap()
    wv_d = nc.dram_tensor("wv", [C, HL * D], BF16, kind="ExternalInput").ap()
    wpt_d = nc.dram_tensor("wpt", [HL * D, C], BF16, kind="ExternalInput").ap()
    bp_d = nc.dram_tensor("bp", [C], F32, kind="ExternalInput").ap()
    y_d = nc.dram_tensor("y", [C, T], F32, kind="ExternalOutput").ap()

    with tile.TileContext(nc) as tc:
        _emit(nc, tc, causal, xT, wq_d, wk_d, wv_d, wpt_d, bp_d, y_d)
    nc.compile()
    return nc


def _emit(nc, tc, causal, xT, wq_d, wk_d, wv_d, wpt_d, bp_d, y_d):
    from contextlib import ExitStack

    ctx = ExitStack()
    with ctx:
        consts = ctx.enter_context(tc.tile_pool(name="consts", bufs=1))
        x_pool = ctx.enter_context(tc.tile_pool(name="xh", bufs=1))
        w_pool = ctx.enter_context(tc.tile_pool(name="w", bufs=1))
        q_pool = ctx.enter_context(tc.tile_pool(name="qT", bufs=3))
        k_pool = ctx.enter_context(tc.tile_pool(name="kT", bufs=3))
        v_pool = ctx.enter_context(tc.tile_pool(name="v", bufs=1))
        oc_pool = ctx.enter_context(tc.tile_pool(name="outcat", bufs=4))
        p_pool = ctx.enter_context(tc.tile_pool(name="pT", bufs=3))
        z_pool = ctx.enter_context(tc.tile_pool(name="zb", bufs=2))
        wpt_pool = ctx.enter_context(tc.tile_pool(name="wpt", bufs=4))
        bpc_pool = ctx.enter_context(tc.tile_pool(name="bpc", bufs=1))
        yst_pool = ctx.enter_context(tc.tile_pool(name="yst", bufs=3))
        psA = ctx.enter_context(tc.tile_pool(name="psA", bufs=2, space="PSUM"))
        psO = ctx.enter_context(tc.tile_pool(name="psO", bufs=2, space="PSUM"))
        psQ = ctx.enter_context(tc.tile_pool(name="psQ", bufs=2, space="PSUM"))

        # ---- constants ----
        # additive causal mask: 0 where free>=partition else -1e9
        mask = None
        if causal:
            mask = consts.tile([P, 2, P], F32)
            nc.vector.memset(mask, 0.0)
            for _u in range(2):
                nc.gpsimd.affine_select(
                    out=mask[:, _u, :], in_=mask[:, _u, :],
                    compare_op=mybir.AluOpType.is_ge,
                    fill=-1e9, base=0,
                    pattern=[[1, P]], channel_multiplier=-1,
                )
        ones_bc = consts.tile([P, P], BF16)
        nc.vector.memset(ones_bc, 1.0)

        # ---- DMA: weights first (first matmuls need them), x in c-chunks ----
        wq_t = w_pool.tile([P, NCH, HL * D], BF16, tag="wq", name="wq")
        wk_t = w_pool.tile([P, NCH, HL * D], BF16, tag="wk", name="wk")
        wv_t = w_pool.tile([P, NCH, HL * D], BF16, tag="wv", name="wv")
        nc.sync.dma_start(out=wq_t, in_=wq_d.rearrange("(n p) d -> p n d", p=P))
        nc.sync.dma_start(out=wk_t, in_=wk_d.rearrange("(n p) d -> p n d", p=P))

        xh = x_pool.tile([P, NCH, T], BF16, tag="xh")
        for c in range(NCH):  # th0 per c-chunk so the first matmuls start early
            nc.sync.dma_start(
                out=xh[:, c, 0:1024], in_=xT[c * P:(c + 1) * P, 0:1024])
        nc.sync.dma_start(out=wv_t, in_=wv_d.rearrange("(n p) d -> p n d", p=P))
        nc.sync.dma_start(
            out=xh[:, :, 1024:2048],
            in_=xT[:, 1024:2048].rearrange("(n p) t -> p n t", p=P))

        wpt_t = [wpt_pool.tile([P, C], BF16, tag="wpt", name=f"wpt{i}")
                 for i in range(4)]
        for q in range(4):
            nc.sync.dma_start(out=wpt_t[q], in_=wpt_d[q * P:(q + 1) * P, :])
        bpc = bpc_pool.tile([P, NCH], F32)
        nc.sync.dma_start(out=bpc, in_=bp_d.rearrange("(n p) -> p n", p=P))

        # ---- persistent activation tiles ----
        qT = [q_pool.tile([P, T], BF16, tag="qT", name=f"qT{i}")
              for i in range(4)]
        kT = [k_pool.tile([P, T], BF16, tag="kT", name=f"kT{i}")
              for i in range(4)]
        # v: [s-part, s-block, head, d + ones]
        v_t = v_pool.tile([P, NSB, HL, D + 1], BF16, tag="v")
        nc.vector.memset(v_t[:, :, :, D:], 1.0)
        outcat = [oc_pool.tile([P, T], BF16, tag="outcat", name=f"outcat{i}")
                  for i in range(4)]

        def qk_q(pr, qq):
            """Project one 512-wide t-quarter of q and k for pair pr."""
            wsl = slice(pr * P, (pr + 1) * P)
            t0 = qq * 512
            for w_t, qkT in ((wq_t, qT), (wk_t, kT)):
                ps = psQ.tile([P, 512], F32, tag="psQ", name="qkps")
                for c in range(NCH):
                    nc.tensor.matmul(
                        ps, w_t[:, c, wsl], xh[:, c, t0:t0 + 512],
                        start=c == 0, stop=c == NCH - 1)
                nc.vector.tensor_copy(out=qkT[pr][:, t0:t0 + 512], in_=ps)

        def v_q(qq):
            """Project v for s-blocks 4qq..4qq+3 (all 8 local heads)."""
            for s in range(4 * qq, 4 * qq + 4):
                vps = psQ.tile([P, 512], F32, tag="psQ", name="vps")
                for c in range(NCH):
                    nc.tensor.matmul(
                        vps, xh[:, c, s * P:(s + 1) * P], wv_t[:, c, :],
                        start=c == 0, stop=c == NCH - 1)
                nc.vector.tensor_copy(
                    out=v_t[:, s:s + 1, :, 0:D],
                    in_=vps.rearrange("p (o h d) -> p o h d", o=1, h=HL))

        def attention(pair, pre_j=None):
            zb = z_pool.tile([P, 3, 512], F32, tag="zb", name=f"zb{pair}")
            for j in range(NTT):
                if pre_j is not None:
                    pre_j(j)
                nsb_j = 4 * (j + 1) if causal else NSB
                outp = [psO.tile([D + 1, 512], F32, tag="psO",
                                 name=f"outp{u}") for u in range(2)]

                def emit_pv(i, lo, last):
                    for u in range(2):
                        nc.tensor.matmul(
                            outp[u][:, lo:512],
                            v_t[:, i, pair * 2 + u, :],
                            pend[i][:, u, lo:512],
                            start=(i == 0), stop=last,
                            skip_group_check=True)
                    del pend[i]

                pend = {}
                prev = None
                for i in range(nsb_j):
                    r = i - 4 * j if causal else -1
                    lo = max(r, 0) * P
                    last = i == nsb_j - 1
                    scs = psA.tile([P, 2, 512], F32, tag="psA", name="scs")
                    pts = p_pool.tile([P, 2, 512], BF16, tag="pT", name="pts")
                    pend[i] = pts
                    for u in range(2):
                        dsl = slice(u * D, (u + 1) * D)
                        nc.tensor.matmul(
                            scs[:, u, lo:512],
                            kT[pair][dsl, i * P:(i + 1) * P],
                            qT[pair][dsl, j * 512 + lo:(j + 1) * 512],
                            start=True, stop=True)
                    if causal and r >= 0:
                        nc.vector.tensor_add(
                            scs[:, :, lo:lo + P], scs[:, :, lo:lo + P], mask)
                    nc.scalar.activation(
                        out=pts[:, :, lo:512], in_=scs[:, :, lo:512],
                        func=mybir.ActivationFunctionType.Exp, scale=SCALE)
                    if prev is not None:
                        emit_pv(*prev)
                    prev = (i, lo, last)
                if prev is not None:
                    emit_pv(*prev)
                for u in range(2):
                    # raw (unnormalized) head output + Z row gather
                    nc.vector.tensor_copy(
                        out=outcat[pair][u * D:(u + 1) * D,
                                         j * 512:(j + 1) * 512],
                        in_=outp[u][0:D, :])
                    k0, slot = _zslot(j, u)
                    nc.vector.tensor_copy(
                        out=zb[k0:k0 + 1, slot, :], in_=outp[u][D:D + 1, :])
            return zb

        def normalize(pair, zb):
            rz = z_pool.tile([P, 3, 512], F32, tag="rz", name="rz")
            nc.vector.reciprocal_approx_fast(out=rz, in_=zb)
            rzb = z_pool.tile([P, 3, 512], BF16, tag="rzb", name="rzb")
            nc.vector.tensor_copy(out=rzb, in_=rz)
            for j in range(NTT):
                for u in range(2):
                    k0, slot = _zslot(j, u)
                    bps = psQ.tile([P, 512], F32, tag="psQ", name="bps")
                    nc.tensor.matmul(
                        bps, ones_bc[k0:k0 + 1, :], rzb[k0:k0 + 1, slot, :],
                        start=True, stop=True)
                    osl = outcat[pair][u * D:(u + 1) * D,
                                       j * 512:(j + 1) * 512]
                    nc.vector.tensor_mul(osl, osl, bps[u * D:(u + 1) * D, :])

        # ---- schedule ----
        # The exp stream on ScalarE is the per-pair bottleneck; feed the PE
        # its own pair's next qk quarter, pair0's v quarters, and the
        # previous pair's normalization inside the attention j-loop.
        zbs = [None] * 4
        if causal:
            qk_q(0, 0)

            def make_pre_j(pr):
                def pre_j(j):
                    if pr == 0:
                        v_q(j)
                    if j < 3:
                        qk_q(pr, j + 1)
                    elif pr < 3:
                        qk_q(pr + 1, 0)
                    if j == 1 and pr > 0:
                        normalize(pr - 1, zbs[pr - 1])
                return pre_j

            for pr in range(4):
                zbs[pr] = attention(pr, pre_j=make_pre_j(pr))
        else:
            # non-causal: every j reads all of kT/v, so project fully first
            for qq in range(4):
                qk_q(0, qq)
                v_q(qq)
            for pr in range(4):
                zbs[pr] = attention(pr)
                if pr < 3:
                    for qq in range(4):
                        qk_q(pr + 1, qq)
                if pr > 0:
                    normalize(pr - 1, zbs[pr - 1])
        normalize(3, zbs[3])

        # ---- output projection: yT[c', t] = wpt[:, c'].T @ outcat[:, t] ----
        for tc_ in range(NTT):
            tg = slice(tc_ * 512, (tc_ + 1) * 512)
            for ci in range(NCH):
                yps = psQ.tile([P, 512], F32, tag="psQ", name="yps")
                for q in range(4):
                    nc.tensor.matmul(
                        yps,
                        wpt_t[q][:, ci * P:(ci + 1) * P],
                        outcat[q][:, tg],
                        start=(q == 0), stop=(q == 3))
                yt = yst_pool.tile([P, 512], F32, tag="yst", name="yt")
                nc.scalar.activation(
                    out=yt, in_=yps,
                    func=mybir.ActivationFunctionType.Identity,
                    bias=bpc[:, ci:ci + 1])
                nc.sync.dma_start(
                    out=y_d[ci * P:(ci + 1) * P, tg], in_=yt)


_NC_CACHE = {}
LAST_RESULTS = None


def kernel(x, Wq, Wk, Wv, Wp, bp, is_masked, **_unused):
    global LAST_RESULTS
    from ml_dtypes import bfloat16

    x = np.asarray(x, np.float32)
    Wq = np.asarray(Wq, np.float32)
    Wk = np.asarray(Wk, np.float32)
    Wv = np.asarray(Wv, np.float32)
    Wp = np.asarray(Wp, np.float32)
    bp = np.asarray(bp, np.float32)
    causal = bool(np.asarray(is_masked).item())

    if causal not in _NC_CACHE:
        _NC_CACHE[causal] = _build(causal)
    nc = _NC_CACHE[causal]

    # host-side layout prep
    wq_r = np.ascontiguousarray(Wq.transpose(1, 0, 2).reshape(C, H * D))
    wk_r = np.ascontiguousarray(Wk.transpose(1, 0, 2).reshape(C, H * D))
    wv_r = np.ascontiguousarray(Wv.transpose(1, 0, 2).reshape(C, H * D))
    wpt = np.ascontiguousarray(Wp.T)
    zeros = np.zeros_like(bp)

    xTs = [np.ascontiguousarray(x[b].T).astype(bfloat16) for b in range(B)]
    in_maps = []
    for core in range(8):
        b, hh = core // 2, core % 2
        csl = slice(hh * HL * D, (hh + 1) * HL * D)
        in_maps.append({
            "xT": xTs[b],
            "wq": np.ascontiguousarray(wq_r[:, csl]).astype(bfloat16),
            "wk": np.ascontiguousarray(wk_r[:, csl]).astype(bfloat16),
            "wv": np.ascontiguousarray(wv_r[:, csl]).astype(bfloat16),
            "wpt": np.ascontiguousarray(wpt[csl, :]).astype(bfloat16),
            "bp": bp if hh == 0 else zeros,
        })

    trace = bool(int(os.environ.get("KERNEL_TRACE", "0")))
    res = run_bass_kernel_spmd(
        nc, in_maps, core_ids=list(range(8)), trace=trace)
    LAST_RESULTS = res

    y = np.empty((B, T, C), np.float32)
    for b in range(B):
        y[b] = res.results[2 * b]["y"].T + res.results[2 * b + 1]["y"].T
    return y


# revision 15
# speedup vs baseline: 1.7560x; 1.0874x over previous
"""Multi-head attention (B=4, T=2048, C=1024, H=16, D=64) on 8 TRN2 cores.

Sharding: core i handles batch b=i//2 and the 8 heads of half hh=i%2.
Each core computes its heads' contribution through the row-sharded output
projection -> partial yT [C, T]; host transposes and sums the two partials
per batch.

v3: all matmul operands bf16 (fp32 "HIGH-mode" matmuls run at ~half PE rate
and block FastWeightLoad). The softmax exp on ScalarE (~157us) is the
per-pair bottleneck, so all PE work that is not on the exp critical path
(q/k projection quarters of the SAME pair one t-tile ahead, v projection
s-quarters, the deferred normalization of the PREVIOUS pair) is interleaved
into the attention j-loop where the PE otherwise idles waiting for exp.
Output projection emits yT = wpt_chunk.T @ outcat so the bias add becomes a
per-partition scalar on the (idle by then) ScalarE.

Per-core layouts (host pre-arranged, bf16):
  xT  [C, T]    = x[b].T
  wq/wk/wv [C, 512]  columns = (local head)*64 + d
  wpt [512, C]  rows  = (local head)*64 + d   (= Wp.T row-slice)
  bp  [C] f32   bias on even cores, zeros on odd (summed partials)
Output: yT [C, T] f32 (host transposes).

On-chip dataflow per core:
  qT/kT [128, T] per head-pair via lhsT=w-chunk, rhs=xT-chunk (N=512)
  v     [s, h, d] natural via lhsT=xT s-slice, rhs=wv (N=512, all 8 heads)
  scoresT[s,t]: lhsT=kT s-block [64,128], rhs=qT t-tile [64,<=512],
                2 heads row-tiled (K=64 each, concurrent on the PE array)
  exp on ScalarE PSUM->SBUF bf16 with scale=1/sqrt(C); causal via additive
  -1e9 mask on the straddling 128-blocks
  PV: lhsT=[v ; ones] [128,65] bf16, rhs=pT -> outT [65,512] PSUM per head,
  accumulated over s-blocks; row 64 = softmax normalizer Z
  normalize: reciprocal_approx_fast(Z) -> bf16 -> per-row K=1 matmul
  broadcast -> DVE mult into outcat (deferred one pair)
  yT: lhsT=wpt c'-chunk, rhs=outcat t-chunk; bias via ScalarE Identity
"""

import os
import sys

import numpy as np

for _p in ("/opt/trn_rl_repo", "/root/.axon_site/_ro/trn_rl_repo"):
    if os.path.isdir(_p) and _p not in sys.path:
        sys.path.append(_p)

import concourse.bass as bass
import concourse.bacc as bacc
import concourse.mybir as mybir
import concourse.tile as tile
from concourse.bass_utils import run_bass_kernel_spmd

B, T, C, H, D = 4, 2048, 1024, 16, 64
HL = H // 2          # heads per core
P = 128
NCH = C // P         # 8 c-chunks
NTT = T // 512       # 4 t-tiles of 512
NSB = T // P         # 16 s-blocks of 128
SCALE = 1.0 / 32.0   # 1/sqrt(C)

F32 = mybir.dt.float32
BF16 = mybir.dt.bfloat16

# zb row (j,u) -> idx=2j+u at partition 32*(idx//3), slot idx%3
# (AP base partitions are restricted to {0,32,64})
def _zslot(j, u):
    idx = 2 * j + u
    return 32 * (idx // 3), idx % 3


def _build(causal: bool) -> bass.Bass:
    nc = bacc.Bacc("TRN2", target_bir_lowering=False, debug=False, num_devices=8)

    xT = nc.dram_tensor("xT", [C, T], BF16, kind="ExternalInput").ap()
    wq_d = nc.dram_tensor("wq", [C, HL * D], BF16, kind="ExternalInput").ap()
    wk_d = nc.dram_tensor("wk", [C, HL * D], BF16, kind="ExternalInput").ap()
    wv_d = nc.dram_tensor("wv", [C, HL * D], BF16, kind="ExternalInput").ap()
    wpt_d = nc.dram_tensor("wpt", [HL * D, C], BF16, kind="ExternalInput").ap()
    bp_d = nc.dram_tensor("bp", [C], F32, kind="ExternalInput").ap()
    y_d = nc.dram_tensor("y", [C, T], F32, kind="ExternalOutput").ap()

    with tile.TileContext(nc) as tc:
        _emit(nc, tc, causal, xT, wq_d, wk_d, wv_d, wpt_d, bp_d, y_d)
    nc.compile()
    return nc


def _emit(nc, tc, causal, xT, wq_d, wk_d, wv_d, wpt_d, bp_d, y_d):
    from contextlib import ExitStack

    ctx = ExitStack()
    with ctx:
        consts = ctx.enter_context(tc.tile_pool(name="consts", bufs=1))
        x_pool = ctx.enter_context(tc.tile_pool(name="xh", bufs=1))
        w_pool = ctx.enter_context(tc.tile_pool(name="w", bufs=1))
        q_pool = ctx.enter_context(tc.tile_pool(name="qT", bufs=3))
        k_pool = ctx.enter_context(tc.tile_pool(name="kT", bufs=3))
        v_pool = ctx.enter_context(tc.tile_pool(name="v", bufs=1))
        oc_pool = ctx.enter_context(tc.tile_pool(name="outcat", bufs=4))
        p_pool = ctx.enter_context(tc.tile_pool(name="pT", bufs=4))
        z_pool = ctx.enter_context(tc.tile_pool(name="zb", bufs=2))
        wpt_pool = ctx.enter_context(tc.tile_pool(name="wpt", bufs=4))
        bpc_pool = ctx.enter_context(tc.tile_pool(name="bpc", bufs=1))
        yst_pool = ctx.enter_context(tc.tile_pool(name="yst", bufs=3))
        psA = ctx.enter_context(tc.tile_pool(name="psA", bufs=2, space="PSUM"))
        psO = ctx.enter_context(tc.tile_pool(name="psO", bufs=2, space="PSUM"))
        psQ = ctx.enter_context(tc.tile_pool(name="psQ", bufs=2, space="PSUM"))

        # ---- constants ----
        # additive causal mask: 0 where free>=partition else -1e9
        mask = None
        if causal:
            mask = consts.tile([P, 2, P], F32)
            nc.vector.memset(mask, 0.0)
            for _u in range(2):
                nc.gpsimd.affine_select(
                    out=mask[:, _u, :], in_=mask[:, _u, :],
                    compare_op=mybir.AluOpType.is_ge,
                    fill=-1e9, base=0,
                    pattern=[[1, P]], channel_multiplier=-1,
                )
        ones_bc = consts.tile([P, P], BF16)
        nc.vector.memset(ones_bc, 1.0)

        # ---- DMA: weights first (first matmuls need them), x in c-chunks ----
        wq_t = w_pool.tile([P, NCH, HL * D], BF16, tag="wq", name="wq")
        wk_t = w_pool.tile([P, NCH, HL * D], BF16, tag="wk", name="wk")
        wv_t = w_pool.tile([P, NCH, HL * D], BF16, tag="wv", name="wv")
        nc.sync.dma_start(out=wq_t, in_=wq_d.rearrange("(n p) d -> p n d", p=P))

        xh = x_pool.tile([P, NCH, T], BF16, tag="xh")
        for c in range(NCH):  # th0 per c-chunk so the first matmuls start early
            nc.sync.dma_start(
                out=xh[:, c, 0:1024], in_=xT[c * P:(c + 1) * P, 0:1024])
        nc.sync.dma_start(out=wk_t, in_=wk_d.rearrange("(n p) d -> p n d", p=P))
        nc.sync.dma_start(out=wv_t, in_=wv_d.rearrange("(n p) d -> p n d", p=P))
        nc.sync.dma_start(
            out=xh[:, :, 1024:2048],
            in_=xT[:, 1024:2048].rearrange("(n p) t -> p n t", p=P))

        wpt_t = [wpt_pool.tile([P, C], BF16, tag="wpt", name=f"wpt{i}")
                 for i in range(4)]
        for q in range(4):
            nc.sync.dma_start(out=wpt_t[q], in_=wpt_d[q * P:(q + 1) * P, :])
        bpc = bpc_pool.tile([P, NCH], F32)
        nc.sync.dma_start(out=bpc, in_=bp_d.rearrange("(n p) -> p n", p=P))

        # ---- persistent activation tiles ----
        qT = [q_pool.tile([P, T], BF16, tag="qT", name=f"qT{i}")
              for i in range(4)]
        kT = [k_pool.tile([P, T], BF16, tag="kT", name=f"kT{i}")
              for i in range(4)]
        # v: [s-part, s-block, head, d + ones]
        v_t = v_pool.tile([P, NSB, HL, D + 1], BF16, tag="v")
        nc.vector.memset(v_t[:, :, :, D:], 1.0)
        outcat = [oc_pool.tile([P, T], BF16, tag="outcat", name=f"outcat{i}")
                  for i in range(4)]

        def qk_q(pr, qq):
            """Project one 512-wide t-quarter of q and k for pair pr."""
            wsl = slice(pr * P, (pr + 1) * P)
            t0 = qq * 512
            for w_t, qkT in ((wq_t, qT), (wk_t, kT)):
                ps = psQ.tile([P, 512], F32, tag="psQ", name="qkps")
                for c in range(NCH):
                    nc.tensor.matmul(
                        ps, w_t[:, c, wsl], xh[:, c, t0:t0 + 512],
                        start=c == 0, stop=c == NCH - 1)
                nc.vector.tensor_copy(out=qkT[pr][:, t0:t0 + 512], in_=ps)

        def v_q(qq):
            """Project v for s-blocks 4qq..4qq+3 (all 8 local heads)."""
            for s in range(4 * qq, 4 * qq + 4):
                vps = psQ.tile([P, 512], F32, tag="psQ", name="vps")
                for c in range(NCH):
                    nc.tensor.matmul(
                        vps, xh[:, c, s * P:(s + 1) * P], wv_t[:, c, :],
                        start=c == 0, stop=c == NCH - 1)
                nc.vector.tensor_copy(
                    out=v_t[:, s:s + 1, :, 0:D],
                    in_=vps.rearrange("p (o h d) -> p o h d", o=1, h=HL))

        def attention(pair, pre_j=None):
            zb = z_pool.tile([P, 3, 512], F32, tag="zb", name=f"zb{pair}")
            zbs[pair] = zb  # visible to this pair's own pre_j hooks
            for j in range(NTT):
                if pre_j is not None:
                    pre_j(j)
                nsb_j = 4 * (j + 1) if causal else NSB
                outp = [psO.tile([D + 1, 512], F32, tag="psO",
                                 name=f"outp{u}") for u in range(2)]

                def emit_pv(i, lo, last):
                    for u in range(2):
                        nc.tensor.matmul(
                            outp[u][:, lo:512],
                            v_t[:, i, pair * 2 + u, :],
                            pend[i][:, u, lo:512],
                            start=(i == 0), stop=last,
                            skip_group_check=True)
                    del pend[i]

                pend = {}
                prev = None
                for i in range(nsb_j):
                    r = i - 4 * j if causal else -1
                    lo = max(r, 0) * P
                    last = i == nsb_j - 1
                    scs = psA.tile([P, 2, 512], F32, tag="psA", name="scs")
                    pts = p_pool.tile([P, 2, 512], BF16, tag="pT", name="pts")
                    pend[i] = pts
                    for u in range(2):
                        dsl = slice(u * D, (u + 1) * D)
                        nc.tensor.matmul(
                            scs[:, u, lo:512],
                            kT[pair][dsl, i * P:(i + 1) * P],
                            qT[pair][dsl, j * 512 + lo:(j + 1) * 512],
                            start=True, stop=True)
                    if causal and r >= 0:
                        nc.vector.tensor_add(
                            scs[:, :, lo:lo + P], scs[:, :, lo:lo + P], mask)
                    nc.scalar.activation(
                        out=pts[:, :, lo:512], in_=scs[:, :, lo:512],
                        func=mybir.ActivationFunctionType.Exp, scale=SCALE)
                    if prev is not None:
                        emit_pv(*prev)
                    prev = (i, lo, last)
                if prev is not None:
                    emit_pv(*prev)
                for u in range(2):
                    # raw (unnormalized) head output + Z row gather
                    nc.vector.tensor_copy(
                        out=outcat[pair][u * D:(u + 1) * D,
                                         j * 512:(j + 1) * 512],
                        in_=outp[u][0:D, :])
                    k0, slot = _zslot(j, u)
                    nc.vector.tensor_copy(
                        out=zb[k0:k0 + 1, slot, :], in_=outp[u][D:D + 1, :])
            return zb

        rzbs = [None] * 4

        def recip_z(pair, zb, sl=slice(0, 3)):
            """Part A of normalization: 1/Z (DVE), f32->bf16.

            sl selects zb slots so pair 3 can normalize incrementally as
            its j-tiles finish. Overlapping slots across calls rewrite the
            same values; the scheduler serializes them (WAW) harmlessly.
            """
            if rzbs[pair] is None:
                rzbs[pair] = (
                    z_pool.tile([P, 3, 512], F32, tag="rz", name=f"rz{pair}"),
                    z_pool.tile([P, 3, 512], BF16, tag="rzb",
                                name=f"rzb{pair}"),
                )
            rz, rzb = rzbs[pair]
            nc.vector.reciprocal_approx_fast(out=rz[:, sl, :], in_=zb[:, sl, :])
            nc.vector.tensor_copy(out=rzb[:, sl, :], in_=rz[:, sl, :])

        def bcast_mul(pair, js):
            """Part B: broadcast 1/Z across partitions, scale outcat."""
            rzb = rzbs[pair][1]
            for j in js:
                for u in range(2):
                    k0, slot = _zslot(j, u)
                    bps = psQ.tile([P, 512], F32, tag="psQ", name="bps")
                    nc.tensor.matmul(
                        bps, ones_bc[k0:k0 + 1, :], rzb[k0:k0 + 1, slot, :],
                        start=True, stop=True)
                    osl = outcat[pair][u * D:(u + 1) * D,
                                       j * 512:(j + 1) * 512]
                    nc.vector.tensor_mul(osl, osl, bps[u * D:(u + 1) * D, :])

        def yproj_chunk(tc_):
            """yT[c', t-chunk] = sum_q wpt[q].T @ outcat[q][:, t-chunk]."""
            tg = slice(tc_ * 512, (tc_ + 1) * 512)
            for ci in range(NCH):
                yps = psQ.tile([P, 512], F32, tag="psQ", name="yps")
                for q in range(4):
                    nc.tensor.matmul(
                        yps,
                        wpt_t[q][:, ci * P:(ci + 1) * P],
                        outcat[q][:, tg],
                        start=(q == 0), stop=(q == 3))
                yt = yst_pool.tile([P, 512], F32, tag="yst", name="yt")
                nc.scalar.activation(
                    out=yt, in_=yps,
                    func=mybir.ActivationFunctionType.Identity,
                    bias=bpc[:, ci:ci + 1])
                nc.sync.dma_start(
                    out=y_d[ci * P:(ci + 1) * P, tg], in_=yt)

        # ---- schedule ----
        # The exp stream on ScalarE is the per-pair bottleneck; feed the PE
        # its own pair's next qk quarter, pair0's v quarters, the previous
        # pair's normalization, and (pair 3) the output projection inside
        # the attention j-loop. Normalization is split so the DVE
        # reciprocal (pre_j(1)) is long done before the PE broadcast
        # matmuls (pre_j(2)) need it.
        zbs = [None] * 4
        if causal:
            qk_q(0, 0)

            def make_pre_j(pr):
                def pre_j(j):
                    if pr == 0:
                        v_q(j)
                    if j < 3:
                        qk_q(pr, j + 1)
                    elif pr < 3:
                        qk_q(pr + 1, 0)
                    if pr > 0:
                        if j == 1:
                            recip_z(pr - 1, zbs[pr - 1])
                        elif j == 2:
                            bcast_mul(pr - 1, range(NTT))
                    if pr == 3:
                        # incremental self-normalize + output projection
                        if j == 2:
                            recip_z(3, zbs[3])  # j0+j1 rows ready
                            bcast_mul(3, (0,))
                        elif j == 3:
                            bcast_mul(3, (1,))
                            yproj_chunk(0)
                return pre_j

            for pr in range(4):
                zbs[pr] = attention(pr, pre_j=make_pre_j(pr))
            recip_z(3, zbs[3])  # j2+j3 rows (slot union spans all 3)
            bcast_mul(3, (2,))
            yproj_chunk(1)
            bcast_mul(3, (3,))
            yproj_chunk(2)
            yproj_chunk(3)
        else:
            # non-causal: every j reads all of kT/v, so project fully first
            for qq in range(4):
                qk_q(0, qq)
                v_q(qq)
            for pr in range(4):
                zbs[pr] = attention(pr)
                if pr < 3:
                    for qq in range(4):
                        qk_q(pr + 1, qq)
                if pr > 0:
                    recip_z(pr - 1, zbs[pr - 1])
                    bcast_mul(pr - 1, range(NTT))
            recip_z(3, zbs[3])
            bcast_mul(3, range(NTT))
            for tc_ in range(NTT):
                yproj_chunk(tc_)


_NC_CACHE = {}
LAST_RESULTS = None


def kernel(x, Wq, Wk, Wv, Wp, bp, is_masked, **_unused):
    global LAST_RESULTS
    from ml_dtypes import bfloat16

    x = np.asarray(x, np.float32)
    Wq = np.asarray(Wq, np.float32)
    Wk = np.asarray(Wk, np.float32)
    Wv = np.asarray(Wv, np.float32)
    Wp = np.asarray(Wp, np.float32)
    bp = np.asarray(bp, np.float32)
    causal = bool(np.asarray(is_masked).item())

    if causal not in _NC_CACHE:
        _NC_CACHE[causal] = _build(causal)
    nc = _NC_CACHE[causal]

    # host-side layout prep
    wq_r = np.ascontiguousarray(Wq.transpose(1, 0, 2).reshape(C, H * D))
    wk_r = np.ascontiguousarray(Wk.transpose(1, 0, 2).reshape(C, H * D))
    wv_r = np.ascontiguousarray(Wv.transpose(1, 0, 2).reshape(C, H * D))
    wpt = np.ascontiguousarray(Wp.T)
    zeros = np.zeros_like(bp)

    xTs = [np.ascontiguousarray(x[b].T).astype(bfloat16) for b in range(B)]
    in_maps = []
    for core in range(8):
        b, hh = core // 2, core % 2
        csl = slice(hh * HL * D, (hh + 1) * HL * D)
        in_maps.append({
            "xT": xTs[b],
            "wq": np.ascontiguousarray(wq_r[:, csl]).astype(bfloat16),
            "wk": np.ascontiguousarray(wk_r[:, csl]).astype(bfloat16),
            "wv": np.ascontiguousarray(wv_r[:, csl]).astype(bfloat16),
            "wpt": np.ascontiguousarray(wpt[csl, :]).astype(bfloat16),
            "bp": bp if hh == 0 else zeros,
        })

    trace = bool(int(os.environ.get("KERNEL_TRACE", "0")))
    res = run_bass_kernel_spmd(
        nc, in_maps, core_ids=list(range(8)), trace=trace)
    LAST_RESULTS = res

    y = np.empty((B, T, C), np.float32)
    for b in range(B):
        y[b] = res.results[2 * b]["y"].T + res.results[2 * b + 1]["y"].T
    return y


# revision 20
# speedup vs baseline: 1.7947x; 1.0221x over previous
"""Multi-head attention (B=4, T=2048, C=1024, H=16, D=64) on 8 TRN2 cores.

Sharding: core i handles batch b=i//2 and the 8 heads of half hh=i%2.
Each core computes its heads' contribution through the row-sharded output
projection -> partial yT [C, T]; host transposes and sums the two partials
per batch.

v3: all matmul operands bf16 (fp32 "HIGH-mode" matmuls run at ~half PE rate
and block FastWeightLoad). The softmax exp on ScalarE (~157us) is the
per-pair bottleneck, so all PE work that is not on the exp critical path
(q/k projection quarters of the SAME pair one t-tile ahead, v projection
s-quarters, the deferred normalization of the PREVIOUS pair) is interleaved
into the attention j-loop where the PE otherwise idles waiting for exp.
Output projection emits yT = wpt_chunk.T @ outcat so the bias add becomes a
per-partition scalar on the (idle by then) ScalarE.

Per-core layouts (host pre-arranged, bf16):
  xT  [C, T]    = x[b].T
  wq/wk/wv [C, 512]  columns = (local head)*64 + d
  wpt [512, C]  rows  = (local head)*64 + d   (= Wp.T row-slice)
  bp  [C] f32   bias on even cores, zeros on odd (summed partials)
Output: yT [C, T] f32 (host transposes).

On-chip dataflow per core:
  qT/kT [128, T] per head-pair via lhsT=w-chunk, rhs=xT-chunk (N=512)
  v     [s, h, d] natural via lhsT=xT s-slice, rhs=wv (N=512, all 8 heads)
  scoresT[s,t]: lhsT=kT s-block [64,128], rhs=qT t-tile [64,<=512],
                2 heads row-tiled (K=64 each, concurrent on the PE array)
  exp on ScalarE PSUM->SBUF bf16 with scale=1/sqrt(C); causal via additive
  -1e9 mask on the straddling 128-blocks
  PV: lhsT=[v ; ones] [128,65] bf16, rhs=pT -> outT [65,512] PSUM per head,
  accumulated over s-blocks; row 64 = softmax normalizer Z
  normalize: reciprocal_approx_fast(Z) -> bf16 -> per-row K=1 matmul
  broadcast -> DVE mult into outcat (deferred one pair)
  yT: lhsT=wpt c'-chunk, rhs=outcat t-chunk; bias via ScalarE Identity
"""

import os
import sys

import numpy as np

for _p in ("/opt/trn_rl_repo", "/root/.axon_site/_ro/trn_rl_repo"):
    if os.path.isdir(_p) and _p not in sys.path:
        sys.path.append(_p)

import concourse.bass as bass
import concourse.bacc as bacc
import concourse.mybir as mybir
import concourse.tile as tile
from concourse.bass_utils import run_bass_kernel_spmd

B, T, C, H, D = 4, 2048, 1024, 16, 64
HL = H // 2          # heads per core
P = 128
NCH = C // P         # 8 c-chunks
NTT = T // 512       # 4 t-tiles of 512
NSB = T // P         # 16 s-blocks of 128
SCALE = 1.0 / 32.0   # 1/sqrt(C)

F32 = mybir.dt.float32
BF16 = mybir.dt.bfloat16

# zb row (j,u) -> idx=2j+u at partition 32*(idx//3), slot idx%3
# (AP base partitions are restricted to {0,32,64})
def _zslot(j, u):
    idx = 2 * j + u
    return 32 * (idx // 3), idx % 3


def _build(causal: bool) -> bass.Bass:
    nc = bacc.Bacc("TRN2", target_bir_lowering=False, debug=False, num_devices=8)

    xT = nc.dram_tensor("xT", [C, T], BF16, kind="ExternalInput").ap()
    wq_d = nc.dram_tensor("wq", [C, HL * D], BF16, kind="ExternalInput").ap()
    wk_d = nc.dram_tensor("wk", [C, HL * D], BF16, kind="ExternalInput").ap()
    wv_d = nc.dram_tensor("wv", [C, HL * D], BF16, kind="ExternalInput").ap()
    wpt_d = nc.dram_tensor("wpt", [HL * D, C], BF16, kind="ExternalInput").ap()
    bp_d = nc.dram_tensor("bp", [C], F32, kind="ExternalInput").ap()
    y_d = nc.dram_tensor("y", [C, T], F32, kind="ExternalOutput").ap()

    with tile.TileContext(nc) as tc:
        _emit(nc, tc, causal, xT, wq_d, wk_d, wv_d, wpt_d, bp_d, y_d)
    nc.compile()
    return nc


def _emit(nc, tc, causal, xT, wq_d, wk_d, wv_d, wpt_d, bp_d, y_d):
    from contextlib import ExitStack

    ctx = ExitStack()
    with ctx:
        consts = ctx.enter_context(tc.tile_pool(name="consts", bufs=1))
        x_pool = ctx.enter_context(tc.tile_pool(name="xh", bufs=1))
        w_pool = ctx.enter_context(tc.tile_pool(name="w", bufs=1))
        q_pool = ctx.enter_context(tc.tile_pool(name="qT", bufs=3))
        k_pool = ctx.enter_context(tc.tile_pool(name="kT", bufs=3))
        v_pool = ctx.enter_context(tc.tile_pool(name="v", bufs=1))
        oc_pool = ctx.enter_context(tc.tile_pool(name="outcat", bufs=4))
        p_pool = ctx.enter_context(tc.tile_pool(name="pT", bufs=4))
        z_pool = ctx.enter_context(tc.tile_pool(name="zb", bufs=2))
        wpt_pool = ctx.enter_context(tc.tile_pool(name="wpt", bufs=4))
        bpc_pool = ctx.enter_context(tc.tile_pool(name="bpc", bufs=1))
        yst_pool = ctx.enter_context(tc.tile_pool(name="yst", bufs=3))
        bps_pool = ctx.enter_context(tc.tile_pool(name="bps", bufs=3))
        psA = ctx.enter_context(tc.tile_pool(name="psA", bufs=2, space="PSUM"))
        psO = ctx.enter_context(tc.tile_pool(name="psO", bufs=2, space="PSUM"))
        psQ = ctx.enter_context(tc.tile_pool(name="psQ", bufs=2, space="PSUM"))

        # ---- constants ----
        # additive causal mask: 0 where free>=partition else -1e9
        mask = None
        if causal:
            mask = consts.tile([P, 2, P], F32)
            nc.vector.memset(mask, 0.0)
            for _u in range(2):
                nc.gpsimd.affine_select(
                    out=mask[:, _u, :], in_=mask[:, _u, :],
                    compare_op=mybir.AluOpType.is_ge,
                    fill=-1e9, base=0,
                    pattern=[[1, P]], channel_multiplier=-1,
                )
        ones_bc = consts.tile([P, P], BF16)
        nc.vector.memset(ones_bc, 1.0)

        # ---- DMA: weights first (first matmuls need them), x in c-chunks ----
        wq_t = w_pool.tile([P, NCH, HL * D], BF16, tag="wq", name="wq")
        wk_t = w_pool.tile([P, NCH, HL * D], BF16, tag="wk", name="wk")
        wv_t = w_pool.tile([P, NCH, HL * D], BF16, tag="wv", name="wv")
        nc.sync.dma_start(out=wq_t, in_=wq_d.rearrange("(n p) d -> p n d", p=P))

        xh = x_pool.tile([P, NCH, T], BF16, tag="xh")
        for c in range(NCH):  # first t-quarter per c-chunk: starts matmuls early
            nc.sync.dma_start(
                out=xh[:, c, 0:512], in_=xT[c * P:(c + 1) * P, 0:512])
        nc.sync.dma_start(out=wk_t, in_=wk_d.rearrange("(n p) d -> p n d", p=P))
        for c in range(NCH):
            nc.sync.dma_start(
                out=xh[:, c, 512:1024], in_=xT[c * P:(c + 1) * P, 512:1024])
        nc.sync.dma_start(out=wv_t, in_=wv_d.rearrange("(n p) d -> p n d", p=P))
        nc.sync.dma_start(
            out=xh[:, :, 1024:2048],
            in_=xT[:, 1024:2048].rearrange("(n p) t -> p n t", p=P))

        wpt_t = [wpt_pool.tile([P, C], BF16, tag="wpt", name=f"wpt{i}")
                 for i in range(4)]
        for q in range(4):
            nc.sync.dma_start(out=wpt_t[q], in_=wpt_d[q * P:(q + 1) * P, :])
        bpc = bpc_pool.tile([P, NCH], F32)
        nc.sync.dma_start(out=bpc, in_=bp_d.rearrange("(n p) -> p n", p=P))

        # ---- persistent activation tiles ----
        qT = [q_pool.tile([P, T], BF16, tag="qT", name=f"qT{i}")
              for i in range(4)]
        kT = [k_pool.tile([P, T], BF16, tag="kT", name=f"kT{i}")
              for i in range(4)]
        # v: [s-part, s-block, head, d + ones]
        v_t = v_pool.tile([P, NSB, HL, D + 1], BF16, tag="v")
        nc.vector.memset(v_t[:, :, :, D:], 1.0)
        outcat = [oc_pool.tile([P, T], BF16, tag="outcat", name=f"outcat{i}")
                  for i in range(4)]

        def qk_q(pr, qq):
            """Project one 512-wide t-quarter of q and k for pair pr."""
            wsl = slice(pr * P, (pr + 1) * P)
            t0 = qq * 512
            for w_t, qkT in ((wq_t, qT), (wk_t, kT)):
                ps = psQ.tile([P, 512], F32, tag="psQ", name="qkps")
                for c in range(NCH):
                    nc.tensor.matmul(
                        ps, w_t[:, c, wsl], xh[:, c, t0:t0 + 512],
                        start=c == 0, stop=c == NCH - 1)
                nc.vector.tensor_copy(out=qkT[pr][:, t0:t0 + 512], in_=ps)

        def v_q(qq):
            """Project v for s-blocks 4qq..4qq+3 (all 8 local heads)."""
            for s in range(4 * qq, 4 * qq + 4):
                vps = psQ.tile([P, 512], F32, tag="psQ", name="vps")
                for c in range(NCH):
                    nc.tensor.matmul(
                        vps, xh[:, c, s * P:(s + 1) * P], wv_t[:, c, :],
                        start=c == 0, stop=c == NCH - 1)
                nc.vector.tensor_copy(
                    out=v_t[:, s:s + 1, :, 0:D],
                    in_=vps.rearrange("p (o h d) -> p o h d", o=1, h=HL))

        def attention(pair, pre_j=None):
            zb = z_pool.tile([P, 3, 512], F32, tag="zb", name=f"zb{pair}")
            zbs[pair] = zb  # visible to this pair's own pre_j hooks
            for j in range(NTT):
                if pre_j is not None:
                    pre_j(j)
                nsb_j = 4 * (j + 1) if causal else NSB
                outp = [psO.tile([D + 1, 512], F32, tag="psO",
                                 name=f"outp{u}") for u in range(2)]

                def emit_pv(i, lo, last):
                    for u in range(2):
                        nc.tensor.matmul(
                            outp[u][:, lo:512],
                            v_t[:, i, pair * 2 + u, :],
                            pend[i][:, u, lo:512],
                            start=(i == 0), stop=last,
                            skip_group_check=True)
                    del pend[i]

                pend = {}
                prev = None
                for i in range(nsb_j):
                    r = i - 4 * j if causal else -1
                    lo = max(r, 0) * P
                    last = i == nsb_j - 1
                    scs = psA.tile([P, 2, 512], F32, tag="psA", name="scs")
                    pts = p_pool.tile([P, 2, 512], BF16, tag="pT", name="pts")
                    pend[i] = pts
                    for u in range(2):
                        dsl = slice(u * D, (u + 1) * D)
                        nc.tensor.matmul(
                            scs[:, u, lo:512],
                            kT[pair][dsl, i * P:(i + 1) * P],
                            qT[pair][dsl, j * 512 + lo:(j + 1) * 512],
                            start=True, stop=True)
                    if causal and r >= 0:
                        nc.vector.tensor_add(
                            scs[:, :, lo:lo + P], scs[:, :, lo:lo + P], mask)
                    nc.scalar.activation(
                        out=pts[:, :, lo:512], in_=scs[:, :, lo:512],
                        func=mybir.ActivationFunctionType.Exp, scale=SCALE)
                    if prev is not None:
                        emit_pv(*prev)
                    prev = (i, lo, last)
                if prev is not None:
                    emit_pv(*prev)
                for u in range(2):
                    # raw (unnormalized) head output + Z row gather
                    nc.vector.tensor_copy(
                        out=outcat[pair][u * D:(u + 1) * D,
                                         j * 512:(j + 1) * 512],
                        in_=outp[u][0:D, :])
                    k0, slot = _zslot(j, u)
                    nc.vector.tensor_copy(
                        out=zb[k0:k0 + 1, slot, :], in_=outp[u][D:D + 1, :])
            return zb

        rzbs = [None] * 4

        def recip_z(pair, zb, psl=slice(0, P), sl=slice(0, 3)):
            """Part A of normalization: 1/Z (DVE), f32->bf16.

            psl/sl select the zb region so pair 3 can normalize
            incrementally as its j-tiles finish without a dependency on
            later rows. Overlapping regions across calls rewrite the same
            values; the scheduler serializes them harmlessly.
            """
            if rzbs[pair] is None:
                rzbs[pair] = (
                    z_pool.tile([P, 3, 512], F32, tag="rz", name=f"rz{pair}"),
                    z_pool.tile([P, 3, 512], BF16, tag="rzb",
                                name=f"rzb{pair}"),
                )
            rz, rzb = rzbs[pair]
            nc.vector.reciprocal_approx_fast(
                out=rz[psl, sl, :], in_=zb[psl, sl, :])
            nc.vector.tensor_copy(out=rzb[psl, sl, :], in_=rz[psl, sl, :])

        def bcast_mul(pair, js):
            """Part B: broadcast 1/Z across partitions (K=1 matmul),
            scale outcat."""
            rzb = rzbs[pair][1]
            for j in js:
                for u in range(2):
                    k0, slot = _zslot(j, u)
                    bps = psQ.tile([P, 512], F32, tag="psQ", name="bps")
                    nc.tensor.matmul(
                        bps, ones_bc[k0:k0 + 1, :], rzb[k0:k0 + 1, slot, :],
                        start=True, stop=True)
                    osl = outcat[pair][u * D:(u + 1) * D,
                                       j * 512:(j + 1) * 512]
                    nc.vector.tensor_mul(osl, osl, bps[u * D:(u + 1) * D, :])

        def yproj_chunk(tc_):
            """yT[c', t-chunk] = sum_q wpt[q].T @ outcat[q][:, t-chunk]."""
            tg = slice(tc_ * 512, (tc_ + 1) * 512)
            for ci in range(NCH):
                yps = psQ.tile([P, 512], F32, tag="psQ", name="yps")
                for q in range(4):
                    nc.tensor.matmul(
                        yps,
                        wpt_t[q][:, ci * P:(ci + 1) * P],
                        outcat[q][:, tg],
                        start=(q == 0), stop=(q == 3))
                yt = yst_pool.tile([P, 512], F32, tag="yst", name="yt")
                nc.scalar.activation(
                    out=yt, in_=yps,
                    func=mybir.ActivationFunctionType.Identity,
                    bias=bpc[:, ci:ci + 1])
                nc.sync.dma_start(
                    out=y_d[ci * P:(ci + 1) * P, tg], in_=yt)

        # ---- schedule ----
        # The exp stream on ScalarE is the per-pair bottleneck; feed the PE
        # its own pair's next qk quarter, pair0's v quarters, the previous
        # pair's normalization, and (pair 3) the output projection inside
        # the attention j-loop. Normalization is split so the DVE
        # reciprocal (pre_j(1)) is long done before the PE broadcast
        # matmuls (pre_j(2)) need it.
        zbs = [None] * 4
        if causal:
            qk_q(0, 0)

            def make_pre_j(pr):
                def pre_j(j):
                    if pr == 0:
                        v_q(j)
                    if j < 3:
                        qk_q(pr, j + 1)
                    elif pr < 3:
                        qk_q(pr + 1, 0)
                    if pr > 0:
                        if j == 1:
                            recip_z(pr - 1, zbs[pr - 1])
                        elif j == 2:
                            bcast_mul(pr - 1, range(NTT))
                    if pr == 3:
                        # incremental self-normalize + output projection
                        if j == 2:
                            recip_z(3, zbs[3])  # j0+j1 rows ready
                            bcast_mul(3, (0,))
                        elif j == 3:
                            # j2 rows live at partition 32 only: a
                            # partition-sliced recip avoids waiting on j3
                            recip_z(3, zbs[3], psl=slice(32, 33),
                                    sl=slice(1, 3))
                            bcast_mul(3, (1, 2))
                            yproj_chunk(0)
                return pre_j

            for pr in range(4):
                zbs[pr] = attention(pr, pre_j=make_pre_j(pr))
            yproj_chunk(1)
            recip_z(3, zbs[3], psl=slice(64, 65), sl=slice(0, 2))  # j3 rows
            yproj_chunk(2)
            bcast_mul(3, (3,))
            yproj_chunk(3)
        else:
            # non-causal: every j reads all of kT/v, so project fully first
            for qq in range(4):
                qk_q(0, qq)
                v_q(qq)
            for pr in range(4):
                zbs[pr] = attention(pr)
                if pr < 3:
                    for qq in range(4):
                        qk_q(pr + 1, qq)
                if pr > 0:
                    recip_z(pr - 1, zbs[pr - 1])
                    bcast_mul(pr - 1, range(NTT))
            recip_z(3, zbs[3])
            bcast_mul(3, range(NTT))
            for tc_ in range(NTT):
                yproj_chunk(tc_)


_NC_CACHE = {}
LAST_RESULTS = None


def kernel(x, Wq, Wk, Wv, Wp, bp, is_masked, **_unused):
    global LAST_RESULTS
    from ml_dtypes import bfloat16

    x = np.asarray(x, np.float32)
    Wq = np.asarray(Wq, np.float32)
    Wk = np.asarray(Wk, np.float32)
    Wv = np.asarray(Wv, np.float32)
    Wp = np.asarray(Wp, np.float32)
    bp = np.asarray(bp, np.float32)
    causal = bool(np.asarray(is_masked).item())

    if causal not in _NC_CACHE:
        _NC_CACHE[causal] = _build(causal)
    nc = _NC_CACHE[causal]

    # host-side layout prep
    wq_r = np.ascontiguousarray(Wq.transpose(1, 0, 2).reshape(C, H * D))
    wk_r = np.ascontiguousarray(Wk.transpose(1, 0, 2).reshape(C, H * D))
    wv_r = np.ascontiguousarray(Wv.transpose(1, 0, 2).reshape(C, H * D))
    wpt = np.ascontiguousarray(Wp.T)
    zeros = np.zeros_like(bp)

    xTs = [np.ascontiguousarray(x[b].T).astype(bfloat16) for b in range(B)]
    in_maps = []
    for core in range(8):
        b, hh = core // 2, core % 2
        csl = slice(hh * HL * D, (hh + 1) * HL * D)
        in_maps.append({
            "xT": xTs[b],
            "wq": np.ascontiguousarray(wq_r[:, csl]).astype(bfloat16),
            "wk": np.ascontiguousarray(wk_r[:, csl]).astype(bfloat16),
            "wv": np.ascontiguousarray(wv_r[:, csl]).astype(bfloat16),
            "wpt": np.ascontiguousarray(wpt[csl, :]).astype(bfloat16),
            "bp": bp if hh == 0 else zeros,
        })

    trace = bool(int(os.environ.get("KERNEL_TRACE", "0")))
    res = run_bass_kernel_spmd(
        nc, in_maps, core_ids=list(range(8)), trace=trace)
    LAST_RESULTS = res

    y = np.empty((B, T, C), np.float32)
    for b in range(B):
        y[b] = res.results[2 * b]["y"].T + res.results[2 * b + 1]["y"].T
    return y


# revision 23
# speedup vs baseline: 1.8302x; 1.0198x over previous
"""Multi-head attention (B=4, T=2048, C=1024, H=16, D=64) on 8 TRN2 cores.

Sharding: core i handles batch b=i//2 and the 8 heads of half hh=i%2.
Each core computes its heads' contribution through the row-sharded output
projection -> partial yT [C, T]; host transposes and sums the two partials
per batch.

v3: all matmul operands bf16 (fp32 "HIGH-mode" matmuls run at ~half PE rate
and block FastWeightLoad). The softmax exp on ScalarE (~157us) is the
per-pair bottleneck, so all PE work that is not on the exp critical path
(q/k projection quarters of the SAME pair one t-tile ahead, v projection
s-quarters, the deferred normalization of the PREVIOUS pair) is interleaved
into the attention j-loop where the PE otherwise idles waiting for exp.
Output projection emits yT = wpt_chunk.T @ outcat so the bias add becomes a
per-partition scalar on the (idle by then) ScalarE.

Per-core layouts (host pre-arranged, bf16):
  xT  [C, T]    = x[b].T
  wq/wk/wv [C, 512]  columns = (local head)*64 + d
  wpt [512, C]  rows  = (local head)*64 + d   (= Wp.T row-slice)
  bp  [C] f32   bias on even cores, zeros on odd (summed partials)
Output: yT [C, T] f32 (host transposes).

On-chip dataflow per core:
  qT/kT [128, T] per head-pair via lhsT=w-chunk, rhs=xT-chunk (N=512)
  v     [s, h, d] natural via lhsT=xT s-slice, rhs=wv (N=512, all 8 heads)
  scoresT[s,t]: lhsT=kT s-block [64,128], rhs=qT t-tile [64,<=512],
                2 heads row-tiled (K=64 each, concurrent on the PE array)
  exp on ScalarE PSUM->SBUF bf16 with scale=1/sqrt(C); causal via additive
  -1e9 mask on the straddling 128-blocks
  PV: lhsT=[v ; ones] [128,65] bf16, rhs=pT -> outT [65,512] PSUM per head,
  accumulated over s-blocks; row 64 = softmax normalizer Z
  normalize: reciprocal_approx_fast(Z) -> bf16 -> per-row K=1 matmul
  broadcast -> DVE mult into outcat (deferred one pair)
  yT: lhsT=wpt c'-chunk, rhs=outcat t-chunk; bias via ScalarE Identity
"""

import os
import sys

import numpy as np

for _p in ("/opt/trn_rl_repo", "/root/.axon_site/_ro/trn_rl_repo"):
    if os.path.isdir(_p) and _p not in sys.path:
        sys.path.append(_p)

import concourse.bass as bass
import concourse.bacc as bacc
import concourse.mybir as mybir
import concourse.tile as tile
from concourse.bass_utils import run_bass_kernel_spmd

B, T, C, H, D = 4, 2048, 1024, 16, 64
HL = H // 2          # heads per core
P = 128
NCH = C // P         # 8 c-chunks
NTT = T // 512       # 4 t-tiles of 512
NSB = T // P         # 16 s-blocks of 128
SCALE = 1.0 / 32.0   # 1/sqrt(C)

F32 = mybir.dt.float32
BF16 = mybir.dt.bfloat16

# zb row (j,u) -> idx=2j+u at partition 32*(idx//3), slot idx%3
# (AP base partitions are restricted to {0,32,64})
def _zslot(j, u):
    idx = 2 * j + u
    return 32 * (idx // 3), idx % 3


def _build(causal: bool) -> bass.Bass:
    nc = bacc.Bacc("TRN2", target_bir_lowering=False, debug=False, num_devices=8)

    xT = nc.dram_tensor("xT", [C, T], BF16, kind="ExternalInput").ap()
    wq_d = nc.dram_tensor("wq", [C, HL * D], BF16, kind="ExternalInput").ap()
    wk_d = nc.dram_tensor("wk", [C, HL * D], BF16, kind="ExternalInput").ap()
    wv_d = nc.dram_tensor("wv", [C, HL * D], BF16, kind="ExternalInput").ap()
    wpt_d = nc.dram_tensor("wpt", [HL * D, C], BF16, kind="ExternalInput").ap()
    bp_d = nc.dram_tensor("bp", [C], F32, kind="ExternalInput").ap()
    y_d = nc.dram_tensor("y", [C, T], F32, kind="ExternalOutput").ap()

    with tile.TileContext(nc) as tc:
        _emit(nc, tc, causal, xT, wq_d, wk_d, wv_d, wpt_d, bp_d, y_d)
    nc.compile()
    return nc


def _emit(nc, tc, causal, xT, wq_d, wk_d, wv_d, wpt_d, bp_d, y_d):
    from contextlib import ExitStack

    ctx = ExitStack()
    with ctx:
        consts = ctx.enter_context(tc.tile_pool(name="consts", bufs=1))
        x_pool = ctx.enter_context(tc.tile_pool(name="xh", bufs=1))
        w_pool = ctx.enter_context(tc.tile_pool(name="w", bufs=1))
        q_pool = ctx.enter_context(tc.tile_pool(name="qT", bufs=3))
        k_pool = ctx.enter_context(tc.tile_pool(name="kT", bufs=3))
        v_pool = ctx.enter_context(tc.tile_pool(name="v", bufs=1))
        oc_pool = ctx.enter_context(tc.tile_pool(name="outcat", bufs=4))
        p_pool = ctx.enter_context(tc.tile_pool(name="pT", bufs=4))
        z_pool = ctx.enter_context(tc.tile_pool(name="zb", bufs=2))
        wpt_pool = ctx.enter_context(tc.tile_pool(name="wpt", bufs=4))
        bpc_pool = ctx.enter_context(tc.tile_pool(name="bpc", bufs=1))
        yst_pool = ctx.enter_context(tc.tile_pool(name="yst", bufs=3))
        bps_pool = ctx.enter_context(tc.tile_pool(name="bps", bufs=3))
        psA = ctx.enter_context(tc.tile_pool(name="psA", bufs=2, space="PSUM"))
        psO = ctx.enter_context(tc.tile_pool(name="psO", bufs=2, space="PSUM"))
        psQ = ctx.enter_context(tc.tile_pool(name="psQ", bufs=2, space="PSUM"))

        # ---- constants ----
        # additive causal mask: 0 where free>=partition else -1e9
        mask = None
        if causal:
            mask = consts.tile([P, 2, P], F32)
            nc.vector.memset(mask, 0.0)
            for _u in range(2):
                nc.gpsimd.affine_select(
                    out=mask[:, _u, :], in_=mask[:, _u, :],
                    compare_op=mybir.AluOpType.is_ge,
                    fill=-1e9, base=0,
                    pattern=[[1, P]], channel_multiplier=-1,
                )
        ones_bc = consts.tile([P, P], BF16)
        nc.vector.memset(ones_bc, 1.0)

        # ---- DMA: weights first (first matmuls need them), x in c-chunks ----
        wq_t = w_pool.tile([P, NCH, HL * D], BF16, tag="wq", name="wq")
        wk_t = w_pool.tile([P, NCH, HL * D], BF16, tag="wk", name="wk")
        wv_t = w_pool.tile([P, NCH, HL * D], BF16, tag="wv", name="wv")
        nc.sync.dma_start(out=wq_t, in_=wq_d.rearrange("(n p) d -> p n d", p=P))

        xh = x_pool.tile([P, NCH, T], BF16, tag="xh")
        for c in range(NCH):  # first t-quarter per c-chunk: starts matmuls early
            nc.sync.dma_start(
                out=xh[:, c, 0:512], in_=xT[c * P:(c + 1) * P, 0:512])
        nc.sync.dma_start(out=wk_t, in_=wk_d.rearrange("(n p) d -> p n d", p=P))
        for c in range(NCH):
            nc.sync.dma_start(
                out=xh[:, c, 512:1024], in_=xT[c * P:(c + 1) * P, 512:1024])
        nc.sync.dma_start(out=wv_t, in_=wv_d.rearrange("(n p) d -> p n d", p=P))
        nc.sync.dma_start(
            out=xh[:, :, 1024:2048],
            in_=xT[:, 1024:2048].rearrange("(n p) t -> p n t", p=P))

        wpt_t = [wpt_pool.tile([P, C], BF16, tag="wpt", name=f"wpt{i}")
                 for i in range(4)]
        for q in range(4):
            nc.sync.dma_start(out=wpt_t[q], in_=wpt_d[q * P:(q + 1) * P, :])
        bpc = bpc_pool.tile([P, NCH], F32)
        nc.sync.dma_start(out=bpc, in_=bp_d.rearrange("(n p) -> p n", p=P))

        # ---- persistent activation tiles ----
        qT = [q_pool.tile([P, T], BF16, tag="qT", name=f"qT{i}")
              for i in range(4)]
        kT = [k_pool.tile([P, T], BF16, tag="kT", name=f"kT{i}")
              for i in range(4)]
        # v: [s-part, s-block, head, d + ones]
        v_t = v_pool.tile([P, NSB, HL, D + 1], BF16, tag="v")
        nc.vector.memset(v_t[:, :, :, D:], 1.0)
        outcat = [oc_pool.tile([P, T], BF16, tag="outcat", name=f"outcat{i}")
                  for i in range(4)]

        def qk_half(pr, qq, which):
            """Project one 512-wide t-quarter of q OR k for pair pr."""
            wsl = slice(pr * P, (pr + 1) * P)
            t0 = qq * 512
            w_t, qkT = (wq_t, qT) if which == 0 else (wk_t, kT)
            ps = psQ.tile([P, 512], F32, tag="psQ", name="qkps")
            for c in range(NCH):
                nc.tensor.matmul(
                    ps, w_t[:, c, wsl], xh[:, c, t0:t0 + 512],
                    start=c == 0, stop=c == NCH - 1)
            nc.vector.tensor_copy(out=qkT[pr][:, t0:t0 + 512], in_=ps)

        def qk_q(pr, qq):
            qk_half(pr, qq, 0)
            qk_half(pr, qq, 1)

        def v_s(s):
            """Project v for s-block s (all 8 local heads)."""
            vps = psQ.tile([P, 512], F32, tag="psQ", name="vps")
            for c in range(NCH):
                nc.tensor.matmul(
                    vps, xh[:, c, s * P:(s + 1) * P], wv_t[:, c, :],
                    start=c == 0, stop=c == NCH - 1)
            nc.vector.tensor_copy(
                out=v_t[:, s:s + 1, :, 0:D],
                in_=vps.rearrange("p (o h d) -> p o h d", o=1, h=HL))

        def v_q(qq):
            for s in range(4 * qq, 4 * qq + 4):
                v_s(s)

        def attention(pair, tile_units=None):
            """tile_units: j -> list of ~1-2us PE work closures, dispensed
            evenly across the j-tile's s-blocks so the PE never idles long
            enough to drop its clock while ScalarE chews on exp."""
            zb = z_pool.tile([P, 3, 512], F32, tag="zb", name=f"zb{pair}")
            zbs[pair] = zb  # visible to this pair's own unit closures
            for j in range(NTT):
                units = tile_units(j) if tile_units else []
                nsb_j = 4 * (j + 1) if causal else NSB
                # dispense unit m before s-block floor(m*nsb/M)
                sched = {}
                for m, fn in enumerate(units):
                    sched.setdefault(m * nsb_j // max(len(units), 1),
                                     []).append(fn)
                outp = [psO.tile([D + 1, 512], F32, tag="psO",
                                 name=f"outp{u}") for u in range(2)]

                def emit_pv(i, lo, last):
                    for u in range(2):
                        nc.tensor.matmul(
                            outp[u][:, lo:512],
                            v_t[:, i, pair * 2 + u, :],
                            pend[i][:, u, lo:512],
                            start=(i == 0), stop=last,
                            skip_group_check=True)
                    del pend[i]

                pend = {}
                prev = None
                for i in range(nsb_j):
                    for fn in sched.get(i, ()):
                        fn()
                    r = i - 4 * j if causal else -1
                    lo = max(r, 0) * P
                    last = i == nsb_j - 1
                    scs = psA.tile([P, 2, 512], F32, tag="psA", name="scs")
                    pts = p_pool.tile([P, 2, 512], BF16, tag="pT", name="pts")
                    pend[i] = pts
                    for u in range(2):
                        dsl = slice(u * D, (u + 1) * D)
                        nc.tensor.matmul(
                            scs[:, u, lo:512],
                            kT[pair][dsl, i * P:(i + 1) * P],
                            qT[pair][dsl, j * 512 + lo:(j + 1) * 512],
                            start=True, stop=True)
                    if causal and r >= 0:
                        nc.vector.tensor_add(
                            scs[:, :, lo:lo + P], scs[:, :, lo:lo + P], mask)
                    nc.scalar.activation(
                        out=pts[:, :, lo:512], in_=scs[:, :, lo:512],
                        func=mybir.ActivationFunctionType.Exp, scale=SCALE)
                    if prev is not None:
                        emit_pv(*prev)
                    prev = (i, lo, last)
                if prev is not None:
                    emit_pv(*prev)
                for u in range(2):
                    # raw (unnormalized) head output + Z row gather
                    nc.vector.tensor_copy(
                        out=outcat[pair][u * D:(u + 1) * D,
                                         j * 512:(j + 1) * 512],
                        in_=outp[u][0:D, :])
                    k0, slot = _zslot(j, u)
                    nc.vector.tensor_copy(
                        out=zb[k0:k0 + 1, slot, :], in_=outp[u][D:D + 1, :])
            return zb

        rzbs = [None] * 4

        def recip_z(pair, zb, psl=slice(0, P), sl=slice(0, 3)):
            """Part A of normalization: 1/Z (DVE), f32->bf16.

            psl/sl select the zb region so pair 3 can normalize
            incrementally as its j-tiles finish without a dependency on
            later rows. Overlapping regions across calls rewrite the same
            values; the scheduler serializes them harmlessly.
            """
            if rzbs[pair] is None:
                rzbs[pair] = (
                    z_pool.tile([P, 3, 512], F32, tag="rz", name=f"rz{pair}"),
                    z_pool.tile([P, 3, 512], BF16, tag="rzb",
                                name=f"rzb{pair}"),
                )
            rz, rzb = rzbs[pair]
            nc.vector.reciprocal_approx_fast(
                out=rz[psl, sl, :], in_=zb[psl, sl, :])
            nc.vector.tensor_copy(out=rzb[psl, sl, :], in_=rz[psl, sl, :])

        def bcast_mul_1(pair, j, u):
            """Part B: broadcast 1/Z across partitions (K=1 matmul),
            scale outcat."""
            rzb = rzbs[pair][1]
            k0, slot = _zslot(j, u)
            bps = psQ.tile([P, 512], F32, tag="psQ", name="bps")
            nc.tensor.matmul(
                bps, ones_bc[k0:k0 + 1, :], rzb[k0:k0 + 1, slot, :],
                start=True, stop=True)
            osl = outcat[pair][u * D:(u + 1) * D, j * 512:(j + 1) * 512]
            nc.vector.tensor_mul(osl, osl, bps[u * D:(u + 1) * D, :])

        def bcast_mul(pair, js):
            for j in js:
                for u in range(2):
                    bcast_mul_1(pair, j, u)

        def yproj_ci(tc_, ci):
            """yT[c'-chunk, t-chunk] = sum_q wpt[q].T @ outcat[q]."""
            tg = slice(tc_ * 512, (tc_ + 1) * 512)
            yps = psQ.tile([P, 512], F32, tag="psQ", name="yps")
            for q in range(4):
                nc.tensor.matmul(
                    yps,
                    wpt_t[q][:, ci * P:(ci + 1) * P],
                    outcat[q][:, tg],
                    start=(q == 0), stop=(q == 3))
            yt = yst_pool.tile([P, 512], F32, tag="yst", name="yt")
            nc.scalar.activation(
                out=yt, in_=yps,
                func=mybir.ActivationFunctionType.Identity,
                bias=bpc[:, ci:ci + 1])
            nc.sync.dma_start(out=y_d[ci * P:(ci + 1) * P, tg], in_=yt)

        def yproj_chunk(tc_):
            for ci in range(NCH):
                yproj_ci(tc_, ci)

        # ---- schedule ----
        # The exp stream on ScalarE is the per-pair bottleneck; all other
        # PE work (same pair's next qk quarter, pair0's v s-blocks, the
        # previous pair's normalization, pair3's output projection) is
        # chopped into ~1-2us units dispensed evenly between s-blocks, so
        # the PE stays warm and ScalarE is never starved. v_s(s) units are
        # scheduled so slice s lands before the s-block that consumes it.
        zbs = [None] * 4
        if causal:
            qk_q(0, 0)

            def L(fn, *a):
                return lambda: fn(*a)

            def make_units(pr):
                def units(j):
                    us = []
                    if pr == 0:
                        us += [L(v_s, s) for s in range(4 * j, 4 * j + 4)]
                    if j < 3:
                        us += [L(qk_half, pr, j + 1, 0),
                               L(qk_half, pr, j + 1, 1)]
                    elif pr < 3:
                        us += [L(qk_half, pr + 1, 0, 0),
                               L(qk_half, pr + 1, 0, 1)]
                    if pr > 0:
                        if j == 1:
                            us.append(L(recip_z, pr - 1, zbs[pr - 1]))
                        elif j == 2:
                            us += [L(bcast_mul_1, pr - 1, jj, u)
                                   for jj in range(NTT) for u in range(2)]
                    if pr == 3:
                        # incremental self-normalize + output projection
                        if j == 2:
                            us.append(L(recip_z, 3, zbs[3]))  # j0+j1 rows
                            us += [L(bcast_mul_1, 3, 0, u) for u in range(2)]
                        elif j == 3:
                            # j2 rows live at partition 32 only: the
                            # partition-sliced recip avoids waiting on j3
                            us.append(L(recip_z, 3, zbs[3], slice(32, 33),
                                        slice(1, 3)))
                            us += [L(bcast_mul_1, 3, jj, u)
                                   for jj in (1, 2) for u in range(2)]
                            us += [L(yproj_ci, 0, ci) for ci in range(NCH)]
                    return us
                return units

            for pr in range(4):
                attention(pr, tile_units=make_units(pr))
            yproj_chunk(1)
            recip_z(3, zbs[3], psl=slice(64, 65), sl=slice(0, 2))  # j3 rows
            yproj_chunk(2)
            bcast_mul(3, (3,))
            yproj_chunk(3)
        else:
            # non-causal: every j reads all of kT/v, so project fully first
            for qq in range(4):
                qk_q(0, qq)
                v_q(qq)
            for pr in range(4):
                zbs[pr] = attention(pr)
                if pr < 3:
                    for qq in range(4):
                        qk_q(pr + 1, qq)
                if pr > 0:
                    recip_z(pr - 1, zbs[pr - 1])
                    bcast_mul(pr - 1, range(NTT))
            recip_z(3, zbs[3])
            bcast_mul(3, range(NTT))
            for tc_ in range(NTT):
                yproj_chunk(tc_)


_NC_CACHE = {}
LAST_RESULTS = None


def kernel(x, Wq, Wk, Wv, Wp, bp, is_masked, **_unused):
    global LAST_RESULTS
    from ml_dtypes import bfloat16

    x = np.asarray(x, np.float32)
    Wq = np.asarray(Wq, np.float32)
    Wk = np.asarray(Wk, np.float32)
    Wv = np.asarray(Wv, np.float32)
    Wp = np.asarray(Wp, np.float32)
    bp = np.asarray(bp, np.float32)
    causal = bool(np.asarray(is_masked).item())

    if causal not in _NC_CACHE:
        _NC_CACHE[causal] = _build(causal)
    nc = _NC_CACHE[causal]

    # host-side layout prep
    wq_r = np.ascontiguousarray(Wq.transpose(1, 0, 2).reshape(C, H * D))
    wk_r = np.ascontiguousarray(Wk.transpose(1, 0, 2).reshape(C, H * D))
    wv_r = np.ascontiguousarray(Wv.transpose(1, 0, 2).reshape(C, H * D))
    wpt = np.ascontiguousarray(Wp.T)
    zeros = np.zeros_like(bp)

    xTs = [np.ascontiguousarray(x[b].T).astype(bfloat16) for b in range(B)]
    in_maps = []
    for core in range(8):
        b, hh = core // 2, core % 2
        csl = slice(hh * HL * D, (hh + 1) * HL * D)
        in_maps.append({
            "xT": xTs[b],
            "wq": np.ascontiguousarray(wq_r[:, csl]).astype(bfloat16),
            "wk": np.ascontiguousarray(wk_r[:, csl]).astype(bfloat16),
            "wv": np.ascontiguousarray(wv_r[:, csl]).astype(bfloat16),
            "wpt": np.ascontiguousarray(wpt[csl, :]).astype(bfloat16),
            "bp": bp if hh == 0 else zeros,
        })

    trace = bool(int(os.environ.get("KERNEL_TRACE", "0")))
    res = run_bass_kernel_spmd(
        nc, in_maps, core_ids=list(range(8)), trace=trace)
    LAST_RESULTS = res

    y = np.empty((B, T, C), np.float32)
    for b in range(B):
        y[b] = res.results[2 * b]["y"].T + res.results[2 * b + 1]["y"].T
    return y


# revision 26
# speedup vs baseline: 1.8607x; 1.0167x over previous
"""Multi-head attention (B=4, T=2048, C=1024, H=16, D=64) on 8 TRN2 cores.

Sharding: core i handles batch b=i//2 and the 8 heads of half hh=i%2.
Each core computes its heads' contribution through the row-sharded output
projection -> partial yT [C, T]; host transposes and sums the two partials
per batch.

v3: all matmul operands bf16 (fp32 "HIGH-mode" matmuls run at ~half PE rate
and block FastWeightLoad). The softmax exp on ScalarE (~157us) is the
per-pair bottleneck, so all PE work that is not on the exp critical path
(q/k projection quarters of the SAME pair one t-tile ahead, v projection
s-quarters, the deferred normalization of the PREVIOUS pair) is interleaved
into the attention j-loop where the PE otherwise idles waiting for exp.
Output projection emits yT = wpt_chunk.T @ outcat so the bias add becomes a
per-partition scalar on the (idle by then) ScalarE.

Per-core layouts (host pre-arranged, bf16):
  xT  [C, T]    = x[b].T
  wq/wk/wv [C, 512]  columns = (local head)*64 + d
  wpt [512, C]  rows  = (local head)*64 + d   (= Wp.T row-slice)
  bp  [C] f32   bias on even cores, zeros on odd (summed partials)
Output: yT [C, T] f32 (host transposes).

On-chip dataflow per core:
  qT/kT [128, T] per head-pair via lhsT=w-chunk, rhs=xT-chunk (N=512)
  v     [s, h, d] natural via lhsT=xT s-slice, rhs=wv (N=512, all 8 heads)
  scoresT[s,t]: lhsT=kT s-block [64,128], rhs=qT t-tile [64,<=512],
                2 heads row-tiled (K=64 each, concurrent on the PE array)
  exp on ScalarE PSUM->SBUF bf16 with scale=1/sqrt(C); causal via additive
  -1e9 mask on the straddling 128-blocks
  PV: lhsT=[v ; ones] [128,65] bf16, rhs=pT -> outT [65,512] PSUM per head,
  accumulated over s-blocks; row 64 = softmax normalizer Z
  normalize: reciprocal_approx_fast(Z) -> bf16 -> per-row K=1 matmul
  broadcast -> DVE mult into outcat (deferred one pair)
  yT: lhsT=wpt c'-chunk, rhs=outcat t-chunk; bias via ScalarE Identity
"""

import os
import sys

import numpy as np

for _p in ("/opt/trn_rl_repo", "/root/.axon_site/_ro/trn_rl_repo"):
    if os.path.isdir(_p) and _p not in sys.path:
        sys.path.append(_p)

import concourse.bass as bass
import concourse.bacc as bacc
import concourse.mybir as mybir
import concourse.tile as tile
from concourse.bass_utils import run_bass_kernel_spmd

B, T, C, H, D = 4, 2048, 1024, 16, 64
HL = H // 2          # heads per core
P = 128
NCH = C // P         # 8 c-chunks
NTT = T // 512       # 4 t-tiles of 512
NSB = T // P         # 16 s-blocks of 128
SCALE = 1.0 / 32.0   # 1/sqrt(C)

F32 = mybir.dt.float32
BF16 = mybir.dt.bfloat16

# zb row (j,u) -> idx=2j+u at partition 32*(idx//3), slot idx%3
# (AP base partitions are restricted to {0,32,64})
def _zslot(j, u):
    idx = 2 * j + u
    return 32 * (idx // 3), idx % 3


def _build(causal: bool) -> bass.Bass:
    nc = bacc.Bacc("TRN2", target_bir_lowering=False, debug=False, num_devices=8)

    xT = nc.dram_tensor("xT", [C, T], BF16, kind="ExternalInput").ap()
    wq_d = nc.dram_tensor("wq", [C, HL * D], BF16, kind="ExternalInput").ap()
    wk_d = nc.dram_tensor("wk", [C, HL * D], BF16, kind="ExternalInput").ap()
    wv_d = nc.dram_tensor("wv", [C, HL * D], BF16, kind="ExternalInput").ap()
    wpt_d = nc.dram_tensor("wpt", [HL * D, C], BF16, kind="ExternalInput").ap()
    bp_d = nc.dram_tensor("bp", [C], F32, kind="ExternalInput").ap()
    y_d = nc.dram_tensor("y", [C, T], F32, kind="ExternalOutput").ap()

    with tile.TileContext(nc) as tc:
        _emit(nc, tc, causal, xT, wq_d, wk_d, wv_d, wpt_d, bp_d, y_d)
    nc.compile()
    return nc


def _emit(nc, tc, causal, xT, wq_d, wk_d, wv_d, wpt_d, bp_d, y_d):
    from contextlib import ExitStack

    ctx = ExitStack()
    with ctx:
        consts = ctx.enter_context(tc.tile_pool(name="consts", bufs=1))
        x_pool = ctx.enter_context(tc.tile_pool(name="xh", bufs=1))
        w_pool = ctx.enter_context(tc.tile_pool(name="w", bufs=1))
        q_pool = ctx.enter_context(tc.tile_pool(name="qT", bufs=3))
        k_pool = ctx.enter_context(tc.tile_pool(name="kT", bufs=3))
        v_pool = ctx.enter_context(tc.tile_pool(name="v", bufs=1))
        oc_pool = ctx.enter_context(tc.tile_pool(name="outcat", bufs=4))
        p_pool = ctx.enter_context(tc.tile_pool(name="pT", bufs=4))
        z_pool = ctx.enter_context(tc.tile_pool(name="zb", bufs=2))
        wpt_pool = ctx.enter_context(tc.tile_pool(name="wpt", bufs=4))
        bpc_pool = ctx.enter_context(tc.tile_pool(name="bpc", bufs=1))
        yst_pool = ctx.enter_context(tc.tile_pool(name="yst", bufs=3))
        bps_pool = ctx.enter_context(tc.tile_pool(name="bps", bufs=3))
        psA = ctx.enter_context(tc.tile_pool(name="psA", bufs=2, space="PSUM"))
        psO = ctx.enter_context(tc.tile_pool(name="psO", bufs=2, space="PSUM"))
        psQ = ctx.enter_context(tc.tile_pool(name="psQ", bufs=2, space="PSUM"))

        # ---- constants ----
        # additive causal mask: 0 where free>=partition else -1e9
        mask = None
        if causal:
            mask = consts.tile([P, 2, P], F32)
            nc.vector.memset(mask, 0.0)
            for _u in range(2):
                nc.gpsimd.affine_select(
                    out=mask[:, _u, :], in_=mask[:, _u, :],
                    compare_op=mybir.AluOpType.is_ge,
                    fill=-1e9, base=0,
                    pattern=[[1, P]], channel_multiplier=-1,
                )
        ones_bc = consts.tile([P, P], BF16)
        nc.vector.memset(ones_bc, 1.0)

        # ---- DMA: weights first (first matmuls need them), x in c-chunks ----
        wq_t = w_pool.tile([P, NCH, HL * D], BF16, tag="wq", name="wq")
        wk_t = w_pool.tile([P, NCH, HL * D], BF16, tag="wk", name="wk")
        wv_t = w_pool.tile([P, NCH, HL * D], BF16, tag="wv", name="wv")
        nc.sync.dma_start(out=wq_t, in_=wq_d.rearrange("(n p) d -> p n d", p=P))

        xh = x_pool.tile([P, NCH, T], BF16, tag="xh")
        for c in range(NCH):  # first t-quarter per c-chunk: starts matmuls early
            nc.sync.dma_start(
                out=xh[:, c, 0:512], in_=xT[c * P:(c + 1) * P, 0:512])
        nc.sync.dma_start(out=wk_t, in_=wk_d.rearrange("(n p) d -> p n d", p=P))
        for c in range(NCH):
            nc.sync.dma_start(
                out=xh[:, c, 512:1024], in_=xT[c * P:(c + 1) * P, 512:1024])
        nc.sync.dma_start(out=wv_t, in_=wv_d.rearrange("(n p) d -> p n d", p=P))
        nc.sync.dma_start(
            out=xh[:, :, 1024:2048],
            in_=xT[:, 1024:2048].rearrange("(n p) t -> p n t", p=P))

        wpt_t = [wpt_pool.tile([P, C], BF16, tag="wpt", name=f"wpt{i}")
                 for i in range(4)]
        for q in range(4):
            nc.sync.dma_start(out=wpt_t[q], in_=wpt_d[q * P:(q + 1) * P, :])
        bpc = bpc_pool.tile([P, NCH], F32)
        nc.sync.dma_start(out=bpc, in_=bp_d.rearrange("(n p) -> p n", p=P))

        # ---- persistent activation tiles ----
        qT = [q_pool.tile([P, T], BF16, tag="qT", name=f"qT{i}")
              for i in range(4)]
        kT = [k_pool.tile([P, T], BF16, tag="kT", name=f"kT{i}")
              for i in range(4)]
        # v: [s-part, s-block, head, d + ones]
        v_t = v_pool.tile([P, NSB, HL, D + 1], BF16, tag="v")
        nc.vector.memset(v_t[:, :, :, D:], 1.0)
        outcat = [oc_pool.tile([P, T], BF16, tag="outcat", name=f"outcat{i}")
                  for i in range(4)]

        def qk_half(pr, qq, which):
            """Project one 512-wide t-quarter of q OR k for pair pr."""
            wsl = slice(pr * P, (pr + 1) * P)
            t0 = qq * 512
            w_t, qkT = (wq_t, qT) if which == 0 else (wk_t, kT)
            ps = psQ.tile([P, 512], F32, tag="psQ", name="qkps")
            for c in range(NCH):
                nc.tensor.matmul(
                    ps, w_t[:, c, wsl], xh[:, c, t0:t0 + 512],
                    start=c == 0, stop=c == NCH - 1)
            nc.vector.tensor_copy(out=qkT[pr][:, t0:t0 + 512], in_=ps)

        def qk_q(pr, qq):
            qk_half(pr, qq, 0)
            qk_half(pr, qq, 1)

        def v_s(s):
            """Project v for s-block s (all 8 local heads)."""
            vps = psQ.tile([P, 512], F32, tag="psQ", name="vps")
            for c in range(NCH):
                nc.tensor.matmul(
                    vps, xh[:, c, s * P:(s + 1) * P], wv_t[:, c, :],
                    start=c == 0, stop=c == NCH - 1)
            nc.vector.tensor_copy(
                out=v_t[:, s:s + 1, :, 0:D],
                in_=vps.rearrange("p (o h d) -> p o h d", o=1, h=HL))

        def v_q(qq):
            for s in range(4 * qq, 4 * qq + 4):
                v_s(s)

        def attention(pair, tile_units=None):
            """tile_units: j -> list of ~1-2us PE work closures, dispensed
            evenly across the j-tile's s-blocks so the PE never idles long
            enough to drop its clock while ScalarE chews on exp."""
            zb = z_pool.tile([P, 3, 512], F32, tag="zb", name=f"zb{pair}")
            zbs[pair] = zb  # visible to this pair's own unit closures
            for j in range(NTT):
                units = tile_units(j) if tile_units else []
                nsb_j = 4 * (j + 1) if causal else NSB
                # dispense unit m before s-block floor(m*nsb/M)
                sched = {}
                for m, fn in enumerate(units):
                    sched.setdefault(m * nsb_j // max(len(units), 1),
                                     []).append(fn)
                outp = [psO.tile([D + 1, 512], F32, tag="psO",
                                 name=f"outp{u}") for u in range(2)]

                def emit_pv(i, lo, last):
                    for u in range(2):
                        nc.tensor.matmul(
                            outp[u][:, lo:512],
                            v_t[:, i, pair * 2 + u, :],
                            pend[i][:, u, lo:512],
                            start=(i == 0), stop=last,
                            skip_group_check=True)
                    del pend[i]

                pend = {}
                prev = None
                for i in range(nsb_j):
                    for fn in sched.get(i, ()):
                        fn()
                    r = i - 4 * j if causal else -1
                    lo = max(r, 0) * P
                    last = i == nsb_j - 1
                    scs = psA.tile([P, 2, 512], F32, tag="psA", name="scs")
                    pts = p_pool.tile([P, 2, 512], BF16, tag="pT", name="pts")
                    pend[i] = pts
                    for u in range(2):
                        dsl = slice(u * D, (u + 1) * D)
                        nc.tensor.matmul(
                            scs[:, u, lo:512],
                            kT[pair][dsl, i * P:(i + 1) * P],
                            qT[pair][dsl, j * 512 + lo:(j + 1) * 512],
                            start=True, stop=True)
                    if causal and r >= 0:
                        nc.vector.tensor_add(
                            scs[:, :, lo:lo + P], scs[:, :, lo:lo + P], mask)
                    nc.scalar.activation(
                        out=pts[:, :, lo:512], in_=scs[:, :, lo:512],
                        func=mybir.ActivationFunctionType.Exp, scale=SCALE)
                    if prev is not None:
                        emit_pv(*prev)
                    prev = (i, lo, last)
                if prev is not None:
                    emit_pv(*prev)
                for u in range(2):
                    # raw (unnormalized) head output + Z row gather
                    nc.vector.tensor_copy(
                        out=outcat[pair][u * D:(u + 1) * D,
                                         j * 512:(j + 1) * 512],
                        in_=outp[u][0:D, :])
                    k0, slot = _zslot(j, u)
                    nc.vector.tensor_copy(
                        out=zb[k0:k0 + 1, slot, :], in_=outp[u][D:D + 1, :])
            return zb

        rzbs = [None] * 4

        def recip_z(pair, zb, psl=slice(0, P), sl=slice(0, 3)):
            """Part A of normalization: 1/Z (DVE), f32->bf16.

            psl/sl select the zb region so pair 3 can normalize
            incrementally as its j-tiles finish without a dependency on
            later rows. Overlapping regions across calls rewrite the same
            values; the scheduler serializes them harmlessly.
            """
            if rzbs[pair] is None:
                rzbs[pair] = (
                    z_pool.tile([P, 3, 512], F32, tag="rz", name=f"rz{pair}"),
                    z_pool.tile([P, 3, 512], BF16, tag="rzb",
                                name=f"rzb{pair}"),
                )
            rz, rzb = rzbs[pair]
            nc.vector.reciprocal_approx_fast(
                out=rz[psl, sl, :], in_=zb[psl, sl, :])
            nc.vector.tensor_copy(out=rzb[psl, sl, :], in_=rz[psl, sl, :])

        def bcast_mul_1(pair, j, u):
            """Part B: broadcast 1/Z across partitions (K=1 matmul),
            scale outcat."""
            rzb = rzbs[pair][1]
            k0, slot = _zslot(j, u)
            bps = psQ.tile([P, 512], F32, tag="psQ", name="bps")
            nc.tensor.matmul(
                bps, ones_bc[k0:k0 + 1, :], rzb[k0:k0 + 1, slot, :],
                start=True, stop=True)
            osl = outcat[pair][u * D:(u + 1) * D, j * 512:(j + 1) * 512]
            nc.vector.tensor_mul(osl, osl, bps[u * D:(u + 1) * D, :])

        def bcast_mul(pair, js):
            for j in js:
                for u in range(2):
                    bcast_mul_1(pair, j, u)

        def yproj_ci(tc_, ci):
            """yT[c'-chunk, t-chunk] = sum_q wpt[q].T @ outcat[q]."""
            tg = slice(tc_ * 512, (tc_ + 1) * 512)
            yps = psQ.tile([P, 512], F32, tag="psQ", name="yps")
            for q in range(4):
                nc.tensor.matmul(
                    yps,
                    wpt_t[q][:, ci * P:(ci + 1) * P],
                    outcat[q][:, tg],
                    start=(q == 0), stop=(q == 3))
            yt = yst_pool.tile([P, 512], F32, tag="yst", name="yt")
            nc.scalar.activation(
                out=yt, in_=yps,
                func=mybir.ActivationFunctionType.Identity,
                bias=bpc[:, ci:ci + 1])
            nc.sync.dma_start(out=y_d[ci * P:(ci + 1) * P, tg], in_=yt)

        def yproj_chunk(tc_):
            for ci in range(NCH):
                yproj_ci(tc_, ci)

        def yproj_wide(tc0, ci):
            """Two t-chunks per PSUM tile: halves the identity/DMA count
            on the tail where nothing else is left to overlap."""
            yps = psA.tile([P, 2, 512], F32, tag="psA", name="ypsw")
            for w, tc_ in enumerate((tc0, tc0 + 1)):
                tg = slice(tc_ * 512, (tc_ + 1) * 512)
                for q in range(4):
                    nc.tensor.matmul(
                        yps[:, w, :],
                        wpt_t[q][:, ci * P:(ci + 1) * P],
                        outcat[q][:, tg],
                        start=(q == 0), stop=(q == 3))
            yt = yst_pool.tile([P, 2, 512], F32, tag="ystw", name="ytw")
            nc.scalar.activation(
                out=yt, in_=yps,
                func=mybir.ActivationFunctionType.Identity,
                bias=bpc[:, ci:ci + 1])
            nc.sync.dma_start(
                out=y_d[ci * P:(ci + 1) * P, tc0 * 512:(tc0 + 2) * 512],
                in_=yt.rearrange("p w t -> p (w t)"))

        # ---- schedule ----
        # The exp stream on ScalarE is the per-pair bottleneck; all other
        # PE work (same pair's next qk quarter, pair0's v s-blocks, the
        # previous pair's normalization, pair3's output projection) is
        # chopped into ~1-2us units dispensed evenly between s-blocks, so
        # the PE stays warm and ScalarE is never starved. v_s(s) units are
        # scheduled so slice s lands before the s-block that consumes it.
        zbs = [None] * 4
        if causal:
            qk_q(0, 0)

            def L(fn, *a):
                return lambda: fn(*a)

            def make_units(pr):
                def units(j):
                    us = []
                    if pr == 0:
                        us += [L(v_s, s) for s in range(4 * j, 4 * j + 4)]
                    if j < 3:
                        us += [L(qk_half, pr, j + 1, 0),
                               L(qk_half, pr, j + 1, 1)]
                    elif pr < 3:
                        us += [L(qk_half, pr + 1, 0, 0),
                               L(qk_half, pr + 1, 0, 1)]
                    if pr > 0:
                        if j == 1:
                            us.append(L(recip_z, pr - 1, zbs[pr - 1]))
                        elif j == 2:
                            us += [L(bcast_mul_1, pr - 1, jj, u)
                                   for jj in range(NTT) for u in range(2)]
                    if pr == 3:
                        # incremental self-normalize + output projection
                        if j == 2:
                            us.append(L(recip_z, 3, zbs[3]))  # j0+j1 rows
                            us += [L(bcast_mul_1, 3, 0, u) for u in range(2)]
                        elif j == 3:
                            # j2 rows live at partition 32 only: the
                            # partition-sliced recip avoids waiting on j3
                            us.append(L(recip_z, 3, zbs[3], slice(32, 33),
                                        slice(1, 3)))
                            us += [L(bcast_mul_1, 3, jj, u)
                                   for jj in (1, 2) for u in range(2)]
                            us += [L(yproj_ci, 0, ci) for ci in range(NCH)]
                    return us
                return units

            for pr in range(4):
                attention(pr, tile_units=make_units(pr))
            recip_z(3, zbs[3], psl=slice(64, 65), sl=slice(0, 2))  # j3 rows
            for ci in range(NCH):
                yproj_wide(1, ci)  # t-chunks 1+2; recip3/j3 hides under it
            bcast_mul(3, (3,))
            yproj_chunk(3)
        else:
            # non-causal: every j reads all of kT/v, so project fully first
            for qq in range(4):
                qk_q(0, qq)
                v_q(qq)
            for pr in range(4):
                zbs[pr] = attention(pr)
                if pr < 3:
                    for qq in range(4):
                        qk_q(pr + 1, qq)
                if pr > 0:
                    recip_z(pr - 1, zbs[pr - 1])
                    bcast_mul(pr - 1, range(NTT))
            recip_z(3, zbs[3])
            bcast_mul(3, range(NTT))
            for tc_ in range(NTT):
                yproj_chunk(tc_)


_NC_CACHE = {}
LAST_RESULTS = None


def kernel(x, Wq, Wk, Wv, Wp, bp, is_masked, **_unused):
    global LAST_RESULTS
    from ml_dtypes import bfloat16

    x = np.asarray(x, np.float32)
    Wq = np.asarray(Wq, np.float32)
    Wk = np.asarray(Wk, np.float32)
    Wv = np.asarray(Wv, np.float32)
    Wp = np.asarray(Wp, np.float32)
    bp = np.asarray(bp, np.float32)
    causal = bool(np.asarray(is_masked).item())

    if causal not in _NC_CACHE:
        _NC_CACHE[causal] = _build(causal)
    nc = _NC_CACHE[causal]

    # host-side layout prep
    wq_r = np.ascontiguousarray(Wq.transpose(1, 0, 2).reshape(C, H * D))
    wk_r = np.ascontiguousarray(Wk.transpose(1, 0, 2).reshape(C, H * D))
    wv_r = np.ascontiguousarray(Wv.transpose(1, 0, 2).reshape(C, H * D))
    wpt = np.ascontiguousarray(Wp.T)
    zeros = np.zeros_like(bp)

    xTs = [np.ascontiguousarray(x[b].T).astype(bfloat16) for b in range(B)]
    in_maps = []
    for core in range(8):
        b, hh = core // 2, core % 2
        csl = slice(hh * HL * D, (hh + 1) * HL * D)
        in_maps.append({
            "xT": xTs[b],
            "wq": np.ascontiguousarray(wq_r[:, csl]).astype(bfloat16),
            "wk": np.ascontiguousarray(wk_r[:, csl]).astype(bfloat16),
            "wv": np.ascontiguousarray(wv_r[:, csl]).astype(bfloat16),
            "wpt": np.ascontiguousarray(wpt[csl, :]).astype(bfloat16),
            "bp": bp if hh == 0 else zeros,
        })

    trace = bool(int(os.environ.get("KERNEL_TRACE", "0")))
    res = run_bass_kernel_spmd(
        nc, in_maps, core_ids=list(range(8)), trace=trace)
    LAST_RESULTS = res

    y = np.empty((B, T, C), np.float32)
    for b in range(B):
        y[b] = res.results[2 * b]["y"].T + res.results[2 * b + 1]["y"].T
    return y
